# revision 2
# baseline (speedup 1.0000x reference)
"""2-layer GAT (heads=4, concat=False, ELU between) on 8 Trainium2 cores.

Strategy (see DESIGN.md):
- Project-then-gather: dense phases compute XCAT[n] = [xh(n) fp16 (256) | als(n) f32 | pad]
  (768B rows) for every node; per-edge dma_gather fetches src rows (descriptor-bound,
  bytes are ~free). Same edge indices serve both layers.
- Core c owns dst nodes [c*6272, (c+1)*6272). Edges dst-sorted into 128-node dst-tiles;
  per tile a fixed number of 128-edge chunks (lo/hi split at node 32768 for int16 idx).
- Per chunk: one-hot selD (dst match) built on DVE; PE transpose -> selDT for the
  per-edge ald lookup (ald_e = selDT^T @ ald_tile); attention w = exp(lrelu(als+ald)-12)
  (constant shift cancels in softmax); G rows scaled by w (ACT per-partition scale);
  PSUM-accumulated scatter agg[d,0:256] += selD^T @ Gw with denominators in cols 256:260.
- Head-mean + ELU; h AllGathered between layers; output assembled on host.
"""
import sys
import os

sys.path.insert(0, '/opt/pypackages')
sys.path.insert(0, '/opt/trn_rl_repo')

import numpy as np

import concourse.bacc as bacc
import concourse.mybir as mybir
import concourse.tile as tile
from concourse.bass_utils import run_bass_kernel_spmd

F16 = mybir.dt.float16
F32 = mybir.dt.float32
I16 = mybir.dt.int16

NEG_SLOPE = 0.2
EXP_SHIFT = 0.0


class Cfg:
    def __init__(self, n, n_in, n_hid, n_out, heads, ncores, tiles_per_core,
                 split):
        self.N = n
        self.IN = n_in
        self.H = n_hid
        self.OUT = n_out
        self.HEADS = heads
        self.NCORES = ncores
        self.T = tiles_per_core              # dst-tiles per core
        self.NPC = tiles_per_core * 128      # nodes per core (padded)
        self.NPAD = ncores * self.NPC        # global padded node count
        self.SPLIT = split                   # int16 gather split boundary
        self.ROW = 384                       # fp16 elems per XCAT row (768B)
        self.XH = heads * n_hid              # 256 (=heads*OUT for layer 2)
        assert self.XH == 256 and self.ROW == 384


FULL = Cfg(50000, 128, 64, 64, 4, 8, 49, 32768)


def _wrap16(idx):
    """[n] int array -> [128, n//16] int16 dma_gather layout, replicated x8."""
    n = len(idx)
    assert n % 16 == 0
    base = np.asarray(idx, dtype=np.int16).reshape(n // 16, 16).T  # [16, n/16]
    return np.tile(base, (8, 1))


def host_prep(cfg, edge_index):
    """Build per-core gather indices / dstloc arrays. Returns dict."""
    src = np.asarray(edge_index[0], dtype=np.int64)
    dst = np.asarray(edge_index[1], dtype=np.int64)
    loops = np.arange(cfg.N, dtype=np.int64)
    src = np.concatenate([src, loops])
    dst = np.concatenate([dst, loops])

    core_of = dst // cfg.NPC
    tile_of = (dst % cfg.NPC) // 128

    # per (core, tile): lo/hi edge lists sorted by src
    lists = [[None] * cfg.T for _ in range(cfg.NCORES)]
    c_lo_max = c_hi_max = 1
    order = np.lexsort((src, tile_of, core_of))
    src_s, dst_s = src[order], dst[order]
    core_s, tile_s = core_of[order], tile_of[order]
    # boundaries
    key = core_s * cfg.T + tile_s
    starts = np.searchsorted(key, np.arange(cfg.NCORES * cfg.T), side='left')
    ends = np.searchsorted(key, np.arange(cfg.NCORES * cfg.T), side='right')
    for c in range(cfg.NCORES):
        for t in range(cfg.T):
            k = c * cfg.T + t
            s, e = starts[k], ends[k]
            es, ed = src_s[s:e], dst_s[s:e]
            lo = es < cfg.SPLIT
            lists[c][t] = (es[lo], ed[lo], es[~lo], ed[~lo])
            c_lo_max = max(c_lo_max, (len(es[lo]) + 127) // 128)
            c_hi_max = max(c_hi_max, (len(es[~lo]) + 127) // 128)
    C_lo, C_hi = c_lo_max, c_hi_max
    C = C_lo + C_hi

    gidx = np.zeros((cfg.NCORES, cfg.T, 128, C * 8), dtype=np.int16)
    dstloc = np.full((cfg.NCORES, cfg.T, 128, C), -1.0, dtype=np.float32)
    for c in range(cfg.NCORES):
        for t in range(cfg.T):
            base = (c * cfg.T + t) * 128
            es_lo, ed_lo, es_hi, ed_hi = lists[c][t]
            ilo = np.zeros(C_lo * 128, dtype=np.int64)
            ilo[:len(es_lo)] = es_lo
            ihi = np.zeros(C_hi * 128, dtype=np.int64)
            ihi[:len(es_hi)] = es_hi - cfg.SPLIT
            gidx[c, t, :, :C_lo * 8] = _wrap16(ilo)
            gidx[c, t, :, C_lo * 8:] = _wrap16(ihi)
            dl = np.full((C * 128,), -1.0, dtype=np.float32)
            dl[:len(ed_lo)] = (ed_lo - base).astype(np.float32)
            dl[C_lo * 128:C_lo * 128 + len(ed_hi)] = \
                (ed_hi - base).astype(np.float32)
            dstloc[c, t] = dl.reshape(C, 128).T
    # ald group-gather indices: groups of 16 nodes; per-core 392 -> pad 512
    gpc = cfg.NPC // 16  # groups per core
    aldg = np.zeros((cfg.NCORES, 128, (gpc + 127) // 128 * 8), dtype=np.int16)
    n_ald = ((gpc + 127) // 128) * 128
    for c in range(cfg.NCORES):
        g = np.zeros(n_ald, dtype=np.int64)
        g[:gpc] = c * gpc + np.arange(gpc)
        aldg[c] = _wrap16(g)
    return dict(C_lo=C_lo, C_hi=C_hi, C=C, gidx=gidx, dstloc=dstloc,
                aldg=aldg, n_ald=n_ald)


def _weights_cat(W, a_src, a_dst, heads, ch):
    """[Fin, heads*ch] + [heads, ch]x2 -> fp16 [Fin, heads*ch + 8]."""
    fin = W.shape[0]
    ws = np.einsum('fhc,hc->fh', W.reshape(fin, heads, ch), a_src)
    wd = np.einsum('fhc,hc->fh', W.reshape(fin, heads, ch), a_dst)
    out = np.zeros((fin, heads * ch + 8), dtype=np.float16)
    out[:, :heads * ch] = W.astype(np.float16)
    out[:, heads * ch:heads * ch + heads] = ws.astype(np.float16)
    out[:, heads * ch + heads:heads * ch + 2 * heads] = wd.astype(np.float16)
    return out


def build_kernel(cfg, C_lo, C_hi, n_ald):
    C = C_lo + C_hi
    nc = bacc.Bacc("TRN2", target_bir_lowering=False, debug=False,
                   num_devices=cfg.NCORES, num_swdge_queues=4)
    NP1 = ((cfg.N + 127) // 128) * 128        # XCAT1 rows (50048)
    T_G1 = NP1 // 128                          # global tiles layer 1 (391)
    NP2 = cfg.NPAD                             # XCAT2 rows (50176)
    T_G2 = NP2 // 128                          # 392

    x_in = nc.dram_tensor("x", [cfg.N, cfg.IN], F32, kind="ExternalInput")
    wa1 = nc.dram_tensor("wa1", [cfg.IN, 264], F16, kind="ExternalInput")
    wa2 = nc.dram_tensor("wa2", [cfg.H, 264], F16, kind="ExternalInput")
    mconst = nc.dram_tensor("mconst", [128, 128], F32, kind="ExternalInput")
    ident = nc.dram_tensor("ident", [128, 128], F32, kind="ExternalInput")
    gidx_d = nc.dram_tensor("gidx", [cfg.T, 128, C * 8], I16,
                            kind="ExternalInput")
    dstloc_d = nc.dram_tensor("dstloc", [cfg.T, 128, C], F32,
                              kind="ExternalInput")
    aldg_d = nc.dram_tensor("aldg", [128, n_ald // 16], I16,
                            kind="ExternalInput")
    out_d = nc.dram_tensor("out_slice", [cfg.NPC, cfg.OUT], F32,
                           kind="ExternalOutput")

    with tile.TileContext(nc) as tc:
        with tc.tile_pool(name="dram", bufs=1, space="DRAM") as dpool, \
             tc.tile_pool(name="const", bufs=1) as cpool, \
             tc.tile_pool(name="work", bufs=2) as pool, \
             tc.tile_pool(name="gpool", bufs=2) as gpool, \
             tc.tile_pool(name="gw", bufs=4) as gwpool, \
             tc.tile_pool(name="seld", bufs=C + 2) as sdpool, \
             tc.tile_pool(name="psum", bufs=2, space="PSUM") as psum, \
             tc.tile_pool(name="psA", bufs=2, space="PSUM") as psA, \
             tc.tile_pool(name="psB", bufs=2, space="PSUM") as psB:

            xs16 = dpool.tile([NP1, cfg.IN], F16, name="xs16", uniquify=False)
            xcat1 = dpool.tile([NP1, cfg.ROW], F16, name="xcat1", uniquify=False)
            aldf1 = dpool.tile([NP2, 4], F32, name="aldf1", uniquify=False)
            aldl1 = dpool.tile([n_ald * 16, 4], F32, name="aldl1", uniquify=False)
            h_loc = dpool.tile([cfg.NPC, 128], F32, name="h_loc", uniquify=False)
            h_full = dpool.tile([NP2, 128], F32, name="h_full", uniquify=False,
                                addr_space="Shared")
            h16 = dpool.tile([NP2, 128], F16, name="h16", uniquify=False)
            xcat2 = dpool.tile([NP2, cfg.ROW], F16, name="xcat2", uniquify=False)
            aldf2 = dpool.tile([NP2, 4], F32, name="aldf2", uniquify=False)
            aldl2 = dpool.tile([n_ald * 16, 4], F32, name="aldl2", uniquify=False)

            mconst_sb = cpool.tile([128, 128], F32)
            nc.sync.dma_start(out=mconst_sb[:], in_=mconst[:, :])
            ident_sb = cpool.tile([128, 128], F32)
            nc.sync.dma_start(out=ident_sb[:], in_=ident[:, :])
            wa1_sb = cpool.tile([cfg.IN, 264], F16)
            nc.sync.dma_start(out=wa1_sb[:], in_=wa1[:, :])
            wa2_sb = cpool.tile([cfg.H, 264], F16)
            nc.sync.dma_start(out=wa2_sb[:], in_=wa2[:, :])
            aldg_sb = cpool.tile([128, n_ald // 16], I16)
            nc.sync.dma_start(out=aldg_sb[:], in_=aldg_d[:, :])
            zero_sb = cpool.tile([128, 128], F16)
            nc.gpsimd.memset(zero_sb[:], 0)
            zero_f32 = cpool.tile([128, 8], F32)
            nc.gpsimd.memset(zero_f32[:], 0)
            zero64 = cpool.tile([128, 64], F32)
            nc.gpsimd.memset(zero64[:], 0)

            # ---- stage x -> fp16, zero pad rows ----
            nc.gpsimd.dma_start(
                out=xs16[0:cfg.N, :].flatten(),
                in_=x_in[:, :].flatten())
            if NP1 > cfg.N:
                npad = NP1 - cfg.N
                nc.sync.dma_start(out=xs16[cfg.N:NP1, :],
                                  in_=zero_sb[0:npad, 0:cfg.IN])
            # zero ALDF1 rows beyond layer-1 global tiles
            if NP2 > NP1:
                nc.sync.dma_start(out=aldf1[NP1:NP2, :],
                                  in_=zero_f32[0:NP2 - NP1, 0:4])

            def dense_phase(src16, n_rows, fin, wa_sb, xcat, aldf):
                """src16 [n_rows, 128-col fp16 staging] @ wa -> xcat + aldf."""
                BT = 8  # subtiles per batch
                B = BT * 128
                nb = 0
                bi = 0
                while nb < n_rows:
                    bsz = min(B, n_rows - nb)
                    st = bsz // 128
                    xT = pool.tile([128, B], F16, name=f"xT{id(xcat)}_{bi}",
                                   tag="xT")
                    nc.sync.dma_start(out=xT[:, 0:bsz],
                                      in_=src16[nb:nb + bsz, :],
                                      transpose=True)
                    xc = pool.tile([128, BT, 264], F16,
                                   name=f"xc{id(xcat)}_{bi}", tag="xc")
                    xcf = xc[:].bitcast(F32)  # [128, BT, 132]
                    arow = pool.tile([128, BT, 4], F32,
                                     name=f"ar{id(xcat)}_{bi}", tag="ar")
                    for s in range(st):
                        ps = psA.tile([128, 264], F32, name=f"dps{bi}_{s}",
                                      tag="dps")
                        nc.tensor.matmul(
                            ps[:], xT[0:fin, s * 128:(s + 1) * 128],
                            wa_sb[:], start=True, stop=True)
                        nc.scalar.activation(
                            xc[:, s, 0:256], ps[:, 0:256],
                            mybir.ActivationFunctionType.Copy)
                        nc.vector.tensor_copy(xcf[:, s, 128:132],
                                              ps[:, 256:260])
                        nc.vector.tensor_copy(arow[:, s, :], ps[:, 260:264])
                    nc.sync.dma_start(
                        out=xcat[nb:nb + bsz, 0:264].rearrange(
                            "(s p) d -> p s d", p=128),
                        in_=xc[:, 0:st, :])
                    nc.sync.dma_start(
                        out=aldf[nb:nb + bsz, :].rearrange(
                            "(s p) d -> p s d", p=128),
                        in_=arow[:, 0:st, :])
                    nb += bsz
                    bi += 1

            def ald_gather(aldf, aldl):
                asb = pool.tile([128, n_ald // 128, 64], F32, tag="asb")
                nc.gpsimd.dma_gather(
                    asb[:],
                    aldf[:, :].rearrange("(g k) d -> g (k d)", k=16),
                    aldg_sb[:], n_ald, n_ald, 64, single_packet=False)
                nc.sync.dma_start(
                    out=aldl[:, :].rearrange("(c p j) d -> p c (j d)",
                                             p=128, j=16),
                    in_=asb[:])

            def edge_sweep(xcat, n_rows, aldl, layer):
                for t in range(cfg.T):
                    q = t % 4
                    sfx = f"_{layer}_{t}"
                    idx_t = pool.tile([128, C * 8], I16, name="ix" + sfx,
                                      tag="ix")
                    nc.sync.dma_start(out=idx_t[:], in_=gidx_d[t, :, :])
                    dst_t = pool.tile([128, C], F32, name="dl" + sfx,
                                      tag="dl")
                    nc.sync.dma_start(out=dst_t[:], in_=dstloc_d[t, :, :])
                    ald_t = pool.tile([128, 4], F32, name="at" + sfx,
                                      tag="at")
                    nc.sync.dma_start(out=ald_t[:],
                                      in_=aldl[t * 128:(t + 1) * 128, :])
                    G = gpool.tile([128, C, cfg.ROW], F16, name="G" + sfx,
                                   tag="G")
                    nc.gpsimd.dma_gather(
                        G[:, 0:C_lo, :], xcat[0:cfg.SPLIT, :],
                        idx_t[:, 0:C_lo * 8], C_lo * 128, C_lo * 128,
                        cfg.ROW, single_packet=False, queue_num=q)
                    nc.gpsimd.dma_gather(
                        G[:, C_lo:C, :], xcat[cfg.SPLIT:n_rows, :],
                        idx_t[:, C_lo * 8:], C_hi * 128, C_hi * 128,
                        cfg.ROW, single_packet=False, queue_num=q)
                    Gf = G[:].bitcast(F32)  # [128, C, 192]

                    ald_ps = psB.tile([128, C * 4], F32, name="alp" + sfx,
                                      tag="alp")
                    sel = []
                    for c in range(C):
                        sd = sdpool.tile([128, 128], F32,
                                         name=f"sd{sfx}_{c}", tag="sd")
                        nc.vector.tensor_scalar(
                            sd[:], mconst_sb[:], dst_t[:, c:c + 1], None,
                            mybir.AluOpType.is_equal)
                        sel.append(sd)
                        trp = psum.tile([128, 128], F32,
                                        name=f"tr{sfx}_{c}", tag="tr")
                        nc.tensor.transpose(trp[:], sd[:], ident_sb[:])
                        sdt = pool.tile([128, 128], F32,
                                        name=f"st{sfx}_{c}", tag="st")
                        nc.vector.tensor_copy(sdt[:], trp[:])
                        nc.tensor.matmul(
                            ald_ps[:, c * 4:(c + 1) * 4], sdt[:], ald_t[:],
                            start=True, stop=True)

                    alde = pool.tile([128, C * 4], F32, name="ae" + sfx,
                                     tag="ae")
                    nc.vector.tensor_copy(alde[:], ald_ps[:])
                    alpha = pool.tile([128, C, 4], F32, name="alf" + sfx,
                                      tag="alf")
                    nc.vector.tensor_tensor(
                        out=alpha[:], in0=Gf[:, :, 128:132],
                        in1=alde[:].rearrange("p (c f) -> p c f", f=4),
                        op=mybir.AluOpType.add)
                    # lrelu = max(z, 0.2z); then w = exp(lrelu - 12)
                    alr = pool.tile([128, C, 4], F32, name="alr" + sfx,
                                    tag="alr")
                    nc.vector.tensor_scalar(
                        alr[:], alpha[:], NEG_SLOPE, None,
                        mybir.AluOpType.mult)
                    nc.vector.tensor_tensor(out=alr[:], in0=alr[:],
                                            in1=alpha[:],
                                            op=mybir.AluOpType.max)
                    w32 = pool.tile([128, C * 4], F32, name="w" + sfx,
                                    tag="w")
                    nc.scalar.activation(
                        w32[:].rearrange("p (c f) -> p c f", f=4), alr[:],
                        mybir.ActivationFunctionType.Exp)

                    agg = psB.tile([128, 260], F32, name="agg" + sfx,
                                   tag="agg")
                    for c in range(C):
                        gw = gwpool.tile([128, 264], F32,
                                         name=f"gw{sfx}_{c}", tag="gw")
                        for h in range(4):
                            nc.scalar.activation(
                                gw[:, h * 64:(h + 1) * 64],
                                G[:, c, h * 64:(h + 1) * 64],
                                mybir.ActivationFunctionType.Copy,
                                scale=w32[:, c * 4 + h:c * 4 + h + 1])
                        nc.vector.tensor_copy(gw[:, 256:260],
                                              w32[:, c * 4:(c + 1) * 4])
                        nc.tensor.matmul(
                            agg[:, 0:260], sel[c][:], gw[:, 0:260],
                            start=(c == 0), stop=(c == C - 1),
                            skip_group_check=True)

                    den = pool.tile([128, 4], F32, name="dn" + sfx, tag="dn")
                    nc.vector.tensor_scalar(den[:], agg[:, 256:260], 1e-16,
                                            None, mybir.AluOpType.max)
                    rec = pool.tile([128, 4], F32, name="rc" + sfx, tag="rc")
                    nc.vector.reciprocal(rec[:], den[:])
                    nc.vector.tensor_scalar(rec[:], rec[:], 0.25, None,
                                            mybir.AluOpType.mult)
                    tmp = pool.tile([128, 4, 64], F32, name="tm" + sfx,
                                    tag="tm")
                    for h in range(4):
                        nc.scalar.activation(
                            tmp[:, h, :], agg[:, h * 64:(h + 1) * 64],
                            mybir.ActivationFunctionType.Copy,
                            scale=rec[:, h:h + 1])
                    s0 = pool.tile([128, 128], F32, name="s0" + sfx,
                                   tag="s0")
                    nc.vector.tensor_copy(s0[:, 64:128], zero64[:])
                    nc.vector.tensor_tensor(out=s0[:, 0:64], in0=tmp[:, 0, :],
                                            in1=tmp[:, 1, :],
                                            op=mybir.AluOpType.add)
                    s1 = pool.tile([128, 64], F32, name="s1" + sfx, tag="s1")
                    nc.vector.tensor_tensor(out=s1[:], in0=tmp[:, 2, :],
                                            in1=tmp[:, 3, :],
                                            op=mybir.AluOpType.add)
                    nc.vector.tensor_tensor(out=s0[:, 0:64],
                                            in0=s0[:, 0:64], in1=s1[:],
                                            op=mybir.AluOpType.add)
                    if layer == 1:
                        # ELU(s) = max(s,0) + exp(min(s,0)) - 1
                        ng = pool.tile([128, 64], F32, name="ng" + sfx,
                                       tag="ng")
                        nc.vector.tensor_scalar(ng[:], s0[:, 0:64], 0.0,
                                                None, mybir.AluOpType.min)
                        ex = pool.tile([128, 64], F32, name="ex" + sfx,
                                       tag="ex")
                        nc.scalar.activation(
                            ex[:], ng[:], mybir.ActivationFunctionType.Exp)
                        nc.vector.tensor_scalar(s0[:, 0:64], s0[:, 0:64],
                                                0.0, None,
                                                mybir.AluOpType.max)
                        nc.vector.tensor_tensor(out=s0[:, 0:64],
                                                in0=s0[:, 0:64], in1=ex[:],
                                                op=mybir.AluOpType.add)
                        nc.vector.tensor_scalar(s0[:, 0:64], s0[:, 0:64],
                                                1.0, None,
                                                mybir.AluOpType.subtract)
                        nc.sync.dma_start(
                            out=h_loc[t * 128:(t + 1) * 128, :], in_=s0[:])
                    else:
                        nc.sync.dma_start(
                            out=out_d[t * 128:(t + 1) * 128, :],
                            in_=s0[:, 0:64])

            # ============ layer 1 ============
            dense_phase(xs16, NP1, cfg.IN, wa1_sb, xcat1, aldf1)
            ald_gather(aldf1, aldl1)
            edge_sweep(xcat1, NP1, aldl1, 1)

            # ============ exchange ============
            nc.gpsimd.collective_compute(
                "AllGather", mybir.AluOpType.bypass,
                replica_groups=[list(range(cfg.NCORES))],
                ins=[h_loc.opt()], outs=[h_full.opt()])
            nc.gpsimd.dma_start(
                out=h16[:, :].flatten(),
                in_=h_full[:, :].flatten())

            # ============ layer 2 ============
            dense_phase(h16, NP2, cfg.H, wa2_sb, xcat2, aldf2)
            ald_gather(aldf2, aldl2)
            edge_sweep(xcat2, NP2, aldl2, 2)

    nc.compile()
    return nc


def _run(cfg, inputs, run_fn):
    prep = host_prep(cfg, inputs["edge_index"])
    wa1 = _weights_cat(np.asarray(inputs["W1"], np.float32),
                       np.asarray(inputs["a_src1"], np.float32),
                       np.asarray(inputs["a_dst1"], np.float32),
                       cfg.HEADS, cfg.H)
    wa2 = _weights_cat(np.asarray(inputs["W2"], np.float32),
                       np.asarray(inputs["a_src2"], np.float32),
                       np.asarray(inputs["a_dst2"], np.float32),
                       cfg.HEADS, cfg.OUT)
    mconst = np.tile(np.arange(128, dtype=np.float32)[None, :], (128, 1))
    ident = np.eye(128, dtype=np.float32)
    x = np.ascontiguousarray(np.asarray(inputs["x"], np.float32))

    nc = build_kernel(cfg, prep["C_lo"], prep["C_hi"], prep["n_ald"])
    in_maps = []
    for c in range(cfg.NCORES):
        in_maps.append({
            "x": x, "wa1": wa1, "wa2": wa2, "mconst": mconst, "ident": ident,
            "gidx": prep["gidx"][c], "dstloc": prep["dstloc"][c],
            "aldg": prep["aldg"][c],
        })
    results = run_fn(nc, in_maps)
    out = np.concatenate([results[c]["out_slice"]
                          for c in range(cfg.NCORES)], axis=0)
    return out[:cfg.N]


def kernel(**inputs) -> np.ndarray:
    cfg = FULL

    def run_fn(nc, in_maps):
        res = run_bass_kernel_spmd(
            nc, in_maps, core_ids=list(range(cfg.NCORES)),
            trace=os.environ.get("GAT_TRACE", "0") == "1")
        global LAST_RESULT
        LAST_RESULT = res
        if res.exec_time_ns is not None:
            print(f"HW exec time: {res.exec_time_ns} ns")
        if res.instructions_and_trace is not None:
            print(f"trace path: {res.instructions_and_trace[1]}")
        return res.results

    return _run(cfg, inputs, run_fn)



# revision 8
# speedup vs baseline: 1.8787x; 1.8787x over previous
"""2-layer GAT (heads=4, concat=False, ELU between) on 8 Trainium2 cores — v2.

Design (v2, rewritten from the one-hot-on-DVE baseline):
- Dense phase per layer (redundant on every core): XCAT[n] = [xh fp16 (256) |
  als f32 (16B) | pad] 768B rows for all nodes + ALD[n] (4 fp16) array.
  PSUM 4-bank batches, drain alternates ACT/DVE.
- Core c owns 49 dst blocks of 128 nodes (load-balanced permutation, uniform
  per-slot chunk counts across cores for SPMD). Edges dst-blocked, sorted by
  src, lo/hi split at 32768 for int16 gather indices; exact per-tile chunk
  counts.
- Host-precomputed one-hot scatter matrices: sel [e->dst] and selT [dst->e]
  per 128-edge chunk, loaded by DMA (fp8/fp16), replacing on-device one-hot
  builds + PE transposes.
- Per tile: gather G rows (768B/edge); PE: ald lookup MMs (selT stationary,
  ald_t fp16 moving); alpha = als+ald (DVE); Lrelu+Exp (ACT); paired w fp16
  (DVE); gw = G*w one broadcast TT (DVE, 2x eligible); PE: agg += sel^T@gw
  (+ denominator cols via sel^T@w) accumulated in PSUM; epilogue: head-mean,
  ELU (layer 1) -> h fp16.
- h exchanged via AllGather of [NPC, 64] fp16; layer 2 identical with
  permuted src positions; output reassembled on host.
"""
import sys
import os

sys.path.insert(0, '/opt/pypackages')
sys.path.insert(0, '/opt/trn_rl_repo')

import numpy as np
import ml_dtypes

import concourse.bacc as bacc
import concourse.mybir as mybir
import concourse.tile as tile
from concourse.bass_utils import run_bass_kernel_spmd

F16 = mybir.dt.float16
F32 = mybir.dt.float32
FP8 = mybir.dt.float8e4
I16 = mybir.dt.int16

SEL_FP8 = True          # sel/selT dtype (exact one-hot either way)
SEL_DT = FP8 if SEL_FP8 else F16
SEL_NP = ml_dtypes.float8_e4m3fn if SEL_FP8 else np.float16

NEG_SLOPE = 0.2

N, IN, H, OUT, HEADS = 50000, 128, 64, 64, 4
NCORES = 8
T = 49                   # dst tile slots per core
NPC = T * 128            # 6272 nodes per core (padded)
NP2 = NCORES * NPC       # 50176 permuted rows
NP1 = ((N + 127) // 128) * 128   # 50048 natural rows
NBLK = NP2 // 128        # 392 block slots
SPLIT = 32768
ROW = 384                # fp16 elems per XCAT row (768B)
NALD_G = 256             # ald gather groups of 32 nodes (196 used, padded)
LAST_RESULT = None


def _wrap16(idx):
    """[n] int array (n % 16 == 0) -> [128, n//16] int16 gather idx layout."""
    n = len(idx)
    base = np.asarray(idx, dtype=np.int16).reshape(n // 16, 16).T
    return np.tile(base, (8, 1))


def host_prep(edge_index):
    """Partition/permute dst blocks, build per-core idx + sel arrays.

    Returns dict with per-core arrays and per-tile chunk counts.
    """
    src = np.asarray(edge_index[0], dtype=np.int64)
    dst = np.asarray(edge_index[1], dtype=np.int64)
    loops = np.arange(N, dtype=np.int64)
    src = np.concatenate([src, loops])
    dst = np.concatenate([dst, loops])

    blk = dst // 128                       # natural dst block of each edge
    nblk_nat = (N + 127) // 128            # 391 natural blocks

    # per natural block: chunk cost for balancing (layer-1 split)
    order = np.argsort(blk, kind='stable')
    src_s, dst_s = src[order], dst[order]
    blk_s = blk[order]
    starts = np.searchsorted(blk_s, np.arange(nblk_nat), side='left')
    ends = np.searchsorted(blk_s, np.arange(nblk_nat), side='right')

    cost = np.zeros(nblk_nat, dtype=np.int64)
    for b in range(nblk_nat):
        es = src_s[starts[b]:ends[b]]
        nlo = int((es < SPLIT).sum())
        nhi = len(es) - nlo
        cost[b] = -(-nlo // 128) + (-(-nhi // 128) if nhi else 0)

    # snake-assign blocks (sorted by cost desc) to (slot, core)
    rank = np.argsort(-cost, kind='stable')      # block ids, desc cost
    # slot t gets blocks rank[8t:8t+8]; pad with -1 (empty) to 392
    slot_blocks = np.full((T, NCORES), -1, dtype=np.int64)
    for i, b in enumerate(rank):
        slot_blocks[i // NCORES, i % NCORES] = b

    # permuted position of each node: node in natural block b at offset o
    # -> core c, slot t ->  row (c*T + t)*128 + o
    perm_pos = np.full(NP2, -1, dtype=np.int64)   # by natural padded row
    blk_of_slot = {}
    for t in range(T):
        for c in range(NCORES):
            b = slot_blocks[t, c]
            if b < 0:
                continue
            base_nat = b * 128
            nn = min(128, N - base_nat)
            rows = (c * T + t) * 128 + np.arange(nn)
            perm_pos[base_nat:base_nat + nn] = rows
    node_pos = perm_pos[:N]                        # natural node -> permuted

    # per (core, slot): edge lists for both layers
    # layer 1 src coordinate: natural id; layer 2: permuted position
    src2 = node_pos[src]

    # ald gather indices: 32-node groups; layer 1 groups = natural block
    # rows, layer 2 groups = own permuted rows
    aldg1 = np.zeros((NCORES, 128, NALD_G // 16), dtype=np.int16)
    aldg2 = np.zeros((NCORES, 128, NALD_G // 16), dtype=np.int16)
    for c in range(NCORES):
        g1 = np.zeros(NALD_G, dtype=np.int64)
        g2 = np.zeros(NALD_G, dtype=np.int64)
        for t in range(T):
            b = slot_blocks[t, c]
            bb = b if b >= 0 else 0
            g1[t * 4:t * 4 + 4] = bb * 4 + np.arange(4)
            g2[t * 4:t * 4 + 4] = c * (NPC // 32) + t * 4 + np.arange(4)
        aldg1[c] = _wrap16(g1)
        aldg2[c] = _wrap16(g2)

    res = {
        "slot_blocks": slot_blocks, "node_pos": node_pos,
        "aldg1": aldg1, "aldg2": aldg2,
    }
    for layer, s_coord in ((1, src), (2, src2)):
        c_lo = np.zeros((NCORES, T), dtype=np.int64)
        c_hi = np.zeros((NCORES, T), dtype=np.int64)
        per_tile = [[None] * T for _ in range(NCORES)]
        for t in range(T):
            for c in range(NCORES):
                b = slot_blocks[t, c]
                if b < 0:
                    per_tile[c][t] = (np.zeros(0, np.int64),
                                      np.zeros(0, np.int64),
                                      np.zeros(0, np.int64),
                                      np.zeros(0, np.int64))
                    continue
                s, e = starts[b], ends[b]
                es = s_coord[order][s:e]
                ed = dst_s[s:e] - b * 128      # local dst 0..127
                o2 = np.argsort(es, kind='stable')
                es, ed = es[o2], ed[o2]
                lo = es < SPLIT
                per_tile[c][t] = (es[lo], ed[lo], es[~lo], ed[~lo])
                c_lo[c, t] = -(-len(es[lo]) // 128)
                c_hi[c, t] = -(-len(es[~lo]) // 128) if (~lo).any() else 0
        # uniform across cores per slot
        C_lo_t = c_lo.max(axis=0)
        C_hi_t = c_hi.max(axis=0)
        C_t = C_lo_t + C_hi_t
        totc = int(C_t.sum())
        offs = np.zeros(T + 1, dtype=np.int64)
        offs[1:] = np.cumsum(C_t)

        gidx = np.zeros((NCORES, 128, totc * 8), dtype=np.int16)
        sel = np.zeros((NCORES, 128, totc * 128), dtype=SEL_NP)
        selT = np.zeros((NCORES, 128, totc * 128), dtype=SEL_NP)
        for c in range(NCORES):
            for t in range(T):
                es_lo, ed_lo, es_hi, ed_hi = per_tile[c][t]
                nlo_c, nhi_c = int(C_lo_t[t]), int(C_hi_t[t])
                base = int(offs[t])
                ilo = np.zeros(nlo_c * 128, dtype=np.int64)
                ilo[:len(es_lo)] = es_lo
                ihi = np.zeros(nhi_c * 128, dtype=np.int64)
                ihi[:len(es_hi)] = es_hi - SPLIT
                gidx[c, :, base * 8:(base + nlo_c) * 8] = _wrap16(ilo)
                if nhi_c:
                    gidx[c, :, (base + nlo_c) * 8:(base + C_t[t]) * 8] = \
                        _wrap16(ihi)
                # one-hot sel / selT (edge position within chunk = partition)
                ed_all = np.concatenate([
                    ed_lo,
                    np.full(nlo_c * 128 - len(ed_lo), -1, np.int64),
                    ed_hi,
                    np.full(nhi_c * 128 - len(ed_hi), -1, np.int64)])
                ck = np.arange(C_t[t] * 128) // 128 + base
                ep = np.arange(C_t[t] * 128) % 128
                valid = ed_all >= 0
                sel[c, ep[valid], ck[valid] * 128 + ed_all[valid]] = 1.0
                selT[c, ed_all[valid], ck[valid] * 128 + ep[valid]] = 1.0
        res[f"L{layer}"] = dict(C_lo_t=C_lo_t, C_hi_t=C_hi_t, C_t=C_t,
                                offs=offs, totc=totc, gidx=gidx,
                                sel=sel, selT=selT)
    return res


def _weights_cat(W, a_src, a_dst, heads, ch):
    """[Fin, heads*ch] + [heads, ch]x2 -> fp16 [Fin, heads*ch + 8]."""
    fin = W.shape[0]
    ws = np.einsum('fhc,hc->fh', W.reshape(fin, heads, ch), a_src)
    wd = np.einsum('fhc,hc->fh', W.reshape(fin, heads, ch), a_dst)
    out = np.zeros((fin, heads * ch + 8), dtype=np.float16)
    out[:, :heads * ch] = W.astype(np.float16)
    out[:, heads * ch:heads * ch + heads] = ws.astype(np.float16)
    out[:, heads * ch + heads:heads * ch + 2 * heads] = wd.astype(np.float16)
    return out


def build_kernel(prep):
    nc = bacc.Bacc("TRN2", target_bir_lowering=False, debug=False,
                   num_devices=NCORES, num_swdge_queues=4)
    L1, L2 = prep["L1"], prep["L2"]
    slot_blocks = prep["slot_blocks"]

    x_in = nc.dram_tensor("x", [N, IN], F32, kind="ExternalInput")
    wa1 = nc.dram_tensor("wa1", [IN, 264], F16, kind="ExternalInput")
    wa2 = nc.dram_tensor("wa2", [H, 264], F16, kind="ExternalInput")
    gidx1_d = nc.dram_tensor("gidx1", [128, L1["totc"] * 8], I16,
                             kind="ExternalInput")
    gidx2_d = nc.dram_tensor("gidx2", [128, L2["totc"] * 8], I16,
                             kind="ExternalInput")
    aldg1_d = nc.dram_tensor("aldg1", [128, NALD_G // 16], I16,
                             kind="ExternalInput")
    aldg2_d = nc.dram_tensor("aldg2", [128, NALD_G // 16], I16,
                             kind="ExternalInput")
    sel1_d = nc.dram_tensor("sel1", [128, L1["totc"] * 128], SEL_DT,
                            kind="ExternalInput")
    selT1_d = nc.dram_tensor("selT1", [128, L1["totc"] * 128], SEL_DT,
                             kind="ExternalInput")
    sel2_d = nc.dram_tensor("sel2", [128, L2["totc"] * 128], SEL_DT,
                            kind="ExternalInput")
    selT2_d = nc.dram_tensor("selT2", [128, L2["totc"] * 128], SEL_DT,
                             kind="ExternalInput")
    out_d = nc.dram_tensor("out_slice", [NPC, OUT], F32,
                           kind="ExternalOutput")

    with tile.TileContext(nc) as tc:
        with tc.tile_pool(name="dram", bufs=1, space="DRAM") as dpool, \
             tc.tile_pool(name="const", bufs=1) as cpool, \
             tc.tile_pool(name="dwork", bufs=3) as dwork, \
             tc.tile_pool(name="ework", bufs=2) as ework, \
             tc.tile_pool(name="gpool", bufs=2) as gpool, \
             tc.tile_pool(name="spool", bufs=2) as spool, \
             tc.tile_pool(name="gwpool", bufs=2) as gwpool:

            xs16 = dpool.tile([NP1, IN], F16, name="xs16", uniquify=False)
            xcat1 = dpool.tile([NP1, ROW], F16, name="xcat1", uniquify=False)
            aldf1 = dpool.tile([NP1, 4], F16, name="aldf1", uniquify=False)
            h_loc = dpool.tile([NPC, 128], F16, name="h_loc",
                               uniquify=False)
            h_full = dpool.tile([NP2, 128], F16, name="h_full",
                                uniquify=False, addr_space="Shared")
            xcat2 = dpool.tile([NP2, ROW], F16, name="xcat2", uniquify=False)
            aldf2 = dpool.tile([NP2, 4], F16, name="aldf2", uniquify=False)
            aldl1 = dpool.tile([NALD_G * 32, 4], F16, name="aldl1",
                               uniquify=False)
            aldl2 = dpool.tile([NALD_G * 32, 4], F16, name="aldl2",
                               uniquify=False)

            wa1_sb = cpool.tile([IN, 264], F16)
            nc.sync.dma_start(out=wa1_sb[:], in_=wa1[:, :])
            wa2_sb = cpool.tile([H, 264], F16)
            nc.sync.dma_start(out=wa2_sb[:], in_=wa2[:, :])
            zero_sb = cpool.tile([128, IN], F16)
            nc.gpsimd.memset(zero_sb[:], 0)
            aldg1_sb = cpool.tile([128, NALD_G // 16], I16)
            nc.sync.dma_start(out=aldg1_sb[:], in_=aldg1_d[:, :])
            aldg2_sb = cpool.tile([128, NALD_G // 16], I16)
            nc.sync.dma_start(out=aldg2_sb[:], in_=aldg2_d[:, :])

            # stage x -> fp16 (dtype-converting DMA), zero pad rows
            nc.gpsimd.dma_start(out=xs16[0:N, :].flatten(),
                                in_=x_in[:, :].flatten())
            if NP1 > N:
                nc.sync.dma_start(out=xs16[N:NP1, :],
                                  in_=zero_sb[0:NP1 - N, :])

            def dense_phase(dps, src16, n_rows, fin, wa_sb, xcat, aldf,
                            lname):
                BT = 4
                nb = 0
                bi = 0
                while nb < n_rows:
                    bsz = min(BT * 128, n_rows - nb)
                    st = bsz // 128
                    sfx = f"_{lname}_{bi}"
                    xT = dwork.tile([fin, BT * 128], F16, name="xT" + sfx,
                                    tag="xT")
                    nc.sync.dma_start(out=xT[:, 0:bsz],
                                      in_=src16[nb:nb + bsz, :],
                                      transpose=True)
                    ps = dps.tile([128, BT, 512], F32, name="dps" + sfx,
                                  tag="dps")
                    for s in range(st):
                        nc.tensor.matmul(
                            ps[:, s, 0:264], xT[:, s * 128:(s + 1) * 128],
                            wa_sb[:], start=True, stop=True)
                    xc = dwork.tile([128, BT, 264], F16, name="xc" + sfx,
                                    tag="xc")
                    if bi % 2 == 0:
                        nc.scalar.activation(
                            xc[:, 0:st, 0:256], ps[:, 0:st, 0:256],
                            mybir.ActivationFunctionType.Copy)
                    else:
                        nc.vector.tensor_copy(xc[:, 0:st, 0:256],
                                              ps[:, 0:st, 0:256])
                    xcf = xc[:].bitcast(F32)       # [128, BT, 132]
                    nc.vector.tensor_copy(xcf[:, 0:st, 128:132],
                                          ps[:, 0:st, 256:260])
                    arow = dwork.tile([128, BT, 4], F16, name="ar" + sfx,
                                      tag="ar")
                    nc.vector.tensor_copy(arow[:, 0:st, :],
                                          ps[:, 0:st, 260:264])
                    nc.sync.dma_start(
                        out=xcat[nb:nb + bsz, 0:264].rearrange(
                            "(s p) d -> p s d", p=128),
                        in_=xc[:, 0:st, :])
                    nc.sync.dma_start(
                        out=aldf[nb:nb + bsz, :].rearrange(
                            "(s p) d -> p s d", p=128),
                        in_=arow[:, 0:st, :])
                    nb += bsz
                    bi += 1

            def ald_stage(aldf, n_rows, aldg_sb, aldl, lname):
                asb = ework.tile([128, NALD_G // 128, 128], F16,
                                 name="asb" + lname, tag="asb")
                nc.gpsimd.dma_gather(
                    asb[:],
                    aldf[:, :].rearrange("(g k) d -> g (k d)", k=32),
                    aldg_sb[:], NALD_G, NALD_G, 128, single_packet=False)
                nc.sync.dma_start(
                    out=aldl[:, :].rearrange("(j p k) d -> p j (k d)",
                                             p=128, k=32),
                    in_=asb[:])

            def tile_front(layer, L, gidx_d, sel_d, selT_d, xcat, n_rows,
                           aldl, psA, t):
                """DMA + ald MMs + alpha/w + gw for tile t. Returns tiles."""
                Ct = int(L["C_t"][t])
                Clo = int(L["C_lo_t"][t])
                base = int(L["offs"][t])
                sfx = f"_{layer}_{t}"
                q = t % 4

                idx_t = ework.tile([128, Ct * 8], I16, name="ix" + sfx,
                                   tag="ix")
                nc.sync.dma_start(out=idx_t[:],
                                  in_=gidx_d[:, base * 8:(base + Ct) * 8])
                sel_t = spool.tile([128, Ct * 128], SEL_DT, name="sl" + sfx,
                                   tag="sl")
                nc.sync.dma_start(
                    out=sel_t[:], in_=sel_d[:, base * 128:(base + Ct) * 128])
                selT_t = spool.tile([128, Ct * 128], SEL_DT, name="sT" + sfx,
                                    tag="sT")
                nc.sync.dma_start(
                    out=selT_t[:],
                    in_=selT_d[:, base * 128:(base + Ct) * 128])
                ald_t = ework.tile([128, 4], F16, name="at" + sfx, tag="at")
                nc.sync.dma_start(out=ald_t[:],
                                  in_=aldl[t * 128:(t + 1) * 128, :])

                G = gpool.tile([128, Ct, ROW], F16, name="G" + sfx, tag="G")
                nc.gpsimd.dma_gather(
                    G[:, 0:Clo, :], xcat[0:SPLIT, :],
                    idx_t[:, 0:Clo * 8], Clo * 128, Clo * 128,
                    ROW, single_packet=False, queue_num=q)
                if Ct > Clo:
                    nc.gpsimd.dma_gather(
                        G[:, Clo:Ct, :], xcat[SPLIT:n_rows, :],
                        idx_t[:, Clo * 8:], (Ct - Clo) * 128,
                        (Ct - Clo) * 128, ROW, single_packet=False,
                        queue_num=q)
                Gf = G[:].bitcast(F32)       # [128, Ct, 192]

                alpha_ps = psA.tile([128, Ct, 4], F32, name="alp" + sfx,
                                    tag="alp")
                for c in range(Ct):
                    nc.tensor.matmul(alpha_ps[:, c, :],
                                     selT_t[:, c * 128:(c + 1) * 128],
                                     ald_t[:], start=True, stop=True)
                alpha = ework.tile([128, Ct, 4], F32, name="alf" + sfx,
                                   tag="alf")
                nc.vector.tensor_tensor(out=alpha[:],
                                        in0=Gf[:, :, 128:132],
                                        in1=alpha_ps[:],
                                        op=mybir.AluOpType.add)
                # w = exp(lrelu(alpha)) = max(exp(alpha), exp(0.2*alpha))
                wa = ework.tile([128, Ct, 4], F32, name="wa" + sfx, tag="wa")
                nc.scalar.activation(wa[:], alpha[:],
                                     mybir.ActivationFunctionType.Exp)
                wb = ework.tile([128, Ct, 4], F32, name="wb" + sfx, tag="wb")
                nc.scalar.activation(wb[:], alpha[:],
                                     mybir.ActivationFunctionType.Exp,
                                     scale=NEG_SLOPE)
                wp = ework.tile([128, Ct, 4, 2], F16, name="wp" + sfx,
                                tag="wp")
                nc.vector.tensor_tensor(
                    out=wp[:],
                    in0=wa[:].unsqueeze(3).broadcast_to([128, Ct, 4, 2]),
                    in1=wb[:].unsqueeze(3).broadcast_to([128, Ct, 4, 2]),
                    op=mybir.AluOpType.max)
                gw = gwpool.tile([128, Ct, 4, 64], F16, name="gw" + sfx,
                                 tag="gw")
                nc.vector.tensor_tensor(
                    out=gw[:].rearrange("p c h (r t) -> p c h r t", t=2),
                    in0=G[:, :, 0:256].rearrange(
                        "p c (h r t) -> p c h r t", h=4, t=2),
                    in1=wp[:].unsqueeze(3).broadcast_to([128, Ct, 4, 32, 2]),
                    op=mybir.AluOpType.mult)
                return sel_t, wp, gw, Ct, sfx

            def tile_back(layer, psB, psD, t, sel_t, wp, gw, Ct, sfx):
                agg = psB.tile([128, 256], F32, name="agg" + sfx, tag="agg")
                dps_t = psD.tile([128, 4], F32, name="dnp" + sfx, tag="dnp")
                for c in range(Ct):
                    nc.tensor.matmul(
                        agg[:, :], sel_t[:, c * 128:(c + 1) * 128],
                        gw[:, c, :, :].rearrange("p h f -> p (h f)"),
                        start=(c == 0), stop=(c == Ct - 1),
                        skip_group_check=True)
                    nc.tensor.matmul(
                        dps_t[:, :], sel_t[:, c * 128:(c + 1) * 128],
                        wp[:, c, :, 0:1].rearrange("p h t -> p (h t)"),
                        start=(c == 0), stop=(c == Ct - 1),
                        skip_group_check=True)
                den = ework.tile([128, 4], F32, name="dn" + sfx, tag="dn")
                nc.vector.tensor_scalar(den[:], dps_t[:], 4.0, None,
                                        mybir.AluOpType.mult)
                rec = ework.tile([128, 4], F32, name="rc" + sfx, tag="rc")
                nc.vector.reciprocal(rec[:], den[:])
                tmp = ework.tile([128, 4, 64], F32, name="tm" + sfx,
                                 tag="tm")
                nc.vector.tensor_tensor(
                    out=tmp[:],
                    in0=agg[:, :].rearrange("p (h f) -> p h f", h=4),
                    in1=rec[:].unsqueeze(2).broadcast_to([128, 4, 64]),
                    op=mybir.AluOpType.mult)
                s2 = ework.tile([128, 2, 64], F32, name="s2" + sfx, tag="s2")
                nc.vector.tensor_tensor(out=s2[:], in0=tmp[:, 0:2, :],
                                        in1=tmp[:, 2:4, :],
                                        op=mybir.AluOpType.add)
                if layer == 1:
                    s1 = ework.tile([128, 64], F32, name="s1" + sfx,
                                    tag="s1")
                    nc.vector.tensor_tensor(out=s1[:], in0=s2[:, 0, :],
                                            in1=s2[:, 1, :],
                                            op=mybir.AluOpType.add)
                    # ELU(s) = max(s,0) - 1 + exp(min(s,0))
                    ng = ework.tile([128, 64], F32, name="ng" + sfx,
                                    tag="ng")
                    nc.vector.tensor_scalar(ng[:], s1[:], 0.0, None,
                                            mybir.AluOpType.min)
                    ex = ework.tile([128, 64], F32, name="ex" + sfx,
                                    tag="ex")
                    nc.scalar.activation(ex[:], ng[:],
                                         mybir.ActivationFunctionType.Exp)
                    pm = ework.tile([128, 64], F32, name="pm" + sfx,
                                    tag="pm")
                    nc.vector.tensor_scalar(pm[:], s1[:], 0.0, 1.0,
                                            mybir.AluOpType.max,
                                            mybir.AluOpType.subtract)
                    hv = ework.tile([128, 128], F16, name="hv" + sfx,
                                    tag="hv")
                    nc.gpsimd.memset(hv[:, 64:128], 0)
                    nc.vector.tensor_tensor(out=hv[:, 0:64], in0=pm[:],
                                            in1=ex[:],
                                            op=mybir.AluOpType.add)
                    nc.sync.dma_start(out=h_loc[t * 128:(t + 1) * 128, :],
                                      in_=hv[:])
                else:
                    s1 = ework.tile([128, 64], F32, name="s1" + sfx,
                                    tag="s1")
                    nc.vector.tensor_tensor(out=s1[:], in0=s2[:, 0, :],
                                            in1=s2[:, 1, :],
                                            op=mybir.AluOpType.add)
                    nc.sync.dma_start(out=out_d[t * 128:(t + 1) * 128, :],
                                      in_=s1[:])

            def edge_sweep(layer, L, gidx_d, sel_d, selT_d, xcat, n_rows,
                           aldl, psA, psB, psD):
                pend = None
                for t in range(T):
                    cur = tile_front(layer, L, gidx_d, sel_d, selT_d, xcat,
                                     n_rows, aldl, psA, t)
                    if pend is not None:
                        tile_back(layer, psB, psD, pend[0], *pend[1])
                    pend = (t, cur)
                tile_back(layer, psB, psD, pend[0], *pend[1])

            # ============ layer 1 ============
            with tc.tile_pool(name="dps1", bufs=2, space="PSUM") as dps:
                dense_phase(dps, xs16, NP1, IN, wa1_sb, xcat1, aldf1, "d1")
            ald_stage(aldf1, NP1, aldg1_sb, aldl1, "a1")
            with tc.tile_pool(name="psA1", bufs=2, space="PSUM") as psA, \
                 tc.tile_pool(name="psB1", bufs=2, space="PSUM") as psB, \
                 tc.tile_pool(name="psD1", bufs=2, space="PSUM") as psD:
                edge_sweep(1, L1, gidx1_d, sel1_d, selT1_d, xcat1, NP1,
                           aldl1, psA, psB, psD)

            # ============ exchange ============
            nc.gpsimd.collective_compute(
                "AllGather", mybir.AluOpType.bypass,
                replica_groups=[list(range(NCORES))],
                ins=[h_loc.opt()], outs=[h_full.opt()])

            # ============ layer 2 ============
            with tc.tile_pool(name="dps2", bufs=2, space="PSUM") as dps:
                dense_phase(dps, h_full, NP2, H, wa2_sb, xcat2, aldf2, "d2")
            ald_stage(aldf2, NP2, aldg2_sb, aldl2, "a2")
            with tc.tile_pool(name="psA2", bufs=2, space="PSUM") as psA, \
                 tc.tile_pool(name="psB2", bufs=2, space="PSUM") as psB, \
                 tc.tile_pool(name="psD2", bufs=2, space="PSUM") as psD:
                edge_sweep(2, L2, gidx2_d, sel2_d, selT2_d, xcat2, NP2,
                           aldl2, psA, psB, psD)

    nc.compile()
    return nc


def kernel(**inputs) -> np.ndarray:
    prep = host_prep(inputs["edge_index"])
    L1, L2 = prep["L1"], prep["L2"]
    wa1 = _weights_cat(np.asarray(inputs["W1"], np.float32),
                       np.asarray(inputs["a_src1"], np.float32),
                       np.asarray(inputs["a_dst1"], np.float32), HEADS, H)
    wa2 = _weights_cat(np.asarray(inputs["W2"], np.float32),
                       np.asarray(inputs["a_src2"], np.float32),
                       np.asarray(inputs["a_dst2"], np.float32), HEADS, OUT)
    x = np.ascontiguousarray(np.asarray(inputs["x"], np.float32))

    nc = build_kernel(prep)
    in_maps = []
    for c in range(NCORES):
        in_maps.append({
            "x": x, "wa1": wa1, "wa2": wa2,
            "gidx1": np.ascontiguousarray(L1["gidx"][c]),
            "aldg1": np.ascontiguousarray(prep["aldg1"][c]),
            "aldg2": np.ascontiguousarray(prep["aldg2"][c]),
            "gidx2": np.ascontiguousarray(L2["gidx"][c]),
            "sel1": np.ascontiguousarray(L1["sel"][c]),
            "selT1": np.ascontiguousarray(L1["selT"][c]),
            "sel2": np.ascontiguousarray(L2["sel"][c]),
            "selT2": np.ascontiguousarray(L2["selT"][c]),
        })

    res = run_bass_kernel_spmd(
        nc, in_maps, core_ids=list(range(NCORES)),
        trace=os.environ.get("GAT_TRACE", "0") == "1")
    global LAST_RESULT
    LAST_RESULT = res
    if res.exec_time_ns is not None:
        print(f"HW exec time: {res.exec_time_ns} ns")
    if res.instructions_and_trace is not None:
        print(f"trace path: {res.instructions_and_trace[1]}")

    # reassemble: permuted rows -> natural order
    full = np.concatenate([res.results[c]["out_slice"]
                           for c in range(NCORES)], axis=0)
    node_pos = prep["node_pos"]
    return full[node_pos].astype(np.float32)


# revision 9
# speedup vs baseline: 2.2064x; 1.1744x over previous
"""2-layer GAT (heads=4, concat=False, ELU between) on 8 Trainium2 cores — v2.

Design (v2, rewritten from the one-hot-on-DVE baseline):
- Dense phase per layer (redundant on every core): XCAT[n] = [xh fp16 (256) |
  als f32 (16B) | pad] 768B rows for all nodes + ALD[n] (4 fp16) array.
  PSUM 4-bank batches, drain alternates ACT/DVE.
- Core c owns 49 dst blocks of 128 nodes (load-balanced permutation, uniform
  per-slot chunk counts across cores for SPMD). Edges dst-blocked, sorted by
  src, lo/hi split at 32768 for int16 gather indices; exact per-tile chunk
  counts.
- Host-precomputed one-hot scatter matrices: sel [e->dst] and selT [dst->e]
  per 128-edge chunk, loaded by DMA (fp8/fp16), replacing on-device one-hot
  builds + PE transposes.
- Per tile: gather G rows (768B/edge); PE: ald lookup MMs (selT stationary,
  ald_t fp16 moving); alpha = als+ald (DVE); Lrelu+Exp (ACT); paired w fp16
  (DVE); gw = G*w one broadcast TT (DVE, 2x eligible); PE: agg += sel^T@gw
  (+ denominator cols via sel^T@w) accumulated in PSUM; epilogue: head-mean,
  ELU (layer 1) -> h fp16.
- h exchanged via AllGather of [NPC, 64] fp16; layer 2 identical with
  permuted src positions; output reassembled on host.
"""
import sys
import os

sys.path.insert(0, '/opt/pypackages')
sys.path.insert(0, '/opt/trn_rl_repo')

import numpy as np
import ml_dtypes

import concourse.bacc as bacc
import concourse.mybir as mybir
import concourse.tile as tile
from concourse.bass_utils import run_bass_kernel_spmd

F16 = mybir.dt.float16
F32 = mybir.dt.float32
FP8 = mybir.dt.float8e4
I16 = mybir.dt.int16

SEL_FP8 = True          # sel/selT dtype (exact one-hot either way)
SEL_DT = FP8 if SEL_FP8 else F16
SEL_NP = ml_dtypes.float8_e4m3fn if SEL_FP8 else np.float16

NEG_SLOPE = 0.2

N, IN, H, OUT, HEADS = 50000, 128, 64, 64, 4
NCORES = 8
T = 49                   # dst tile slots per core
NPC = T * 128            # 6272 nodes per core (padded)
NP2 = NCORES * NPC       # 50176 permuted rows
NP1 = ((N + 127) // 128) * 128   # 50048 natural rows
NBLK = NP2 // 128        # 392 block slots
SPLIT = 32768
ROW = 384                # fp16 elems per XCAT row (768B)
NALD_G = 256             # ald gather groups of 32 nodes (196 used, padded)
LAST_RESULT = None


def _wrap16(idx):
    """[n] int array (n % 16 == 0) -> [128, n//16] int16 gather idx layout."""
    n = len(idx)
    base = np.asarray(idx, dtype=np.int16).reshape(n // 16, 16).T
    return np.tile(base, (8, 1))


def host_prep(edge_index):
    """Partition/permute dst blocks, build per-core idx + sel arrays.

    Returns dict with per-core arrays and per-tile chunk counts.
    """
    src = np.asarray(edge_index[0], dtype=np.int64)
    dst = np.asarray(edge_index[1], dtype=np.int64)
    loops = np.arange(N, dtype=np.int64)
    src = np.concatenate([src, loops])
    dst = np.concatenate([dst, loops])

    blk = dst // 128                       # natural dst block of each edge
    nblk_nat = (N + 127) // 128            # 391 natural blocks

    # per natural block: chunk cost for balancing (layer-1 split)
    order = np.argsort(blk, kind='stable')
    src_s, dst_s = src[order], dst[order]
    blk_s = blk[order]
    starts = np.searchsorted(blk_s, np.arange(nblk_nat), side='left')
    ends = np.searchsorted(blk_s, np.arange(nblk_nat), side='right')

    cost = np.zeros(nblk_nat, dtype=np.int64)
    for b in range(nblk_nat):
        es = src_s[starts[b]:ends[b]]
        nlo = int((es < SPLIT).sum())
        nhi = len(es) - nlo
        cost[b] = -(-nlo // 128) + (-(-nhi // 128) if nhi else 0)

    # snake-assign blocks (sorted by cost desc) to (slot, core)
    rank = np.argsort(-cost, kind='stable')      # block ids, desc cost
    # slot t gets blocks rank[8t:8t+8]; pad with -1 (empty) to 392
    slot_blocks = np.full((T, NCORES), -1, dtype=np.int64)
    for i, b in enumerate(rank):
        slot_blocks[i // NCORES, i % NCORES] = b

    # permuted position of each node: node in natural block b at offset o
    # -> core c, slot t ->  row (c*T + t)*128 + o
    perm_pos = np.full(NP2, -1, dtype=np.int64)   # by natural padded row
    blk_of_slot = {}
    for t in range(T):
        for c in range(NCORES):
            b = slot_blocks[t, c]
            if b < 0:
                continue
            base_nat = b * 128
            nn = min(128, N - base_nat)
            rows = (c * T + t) * 128 + np.arange(nn)
            perm_pos[base_nat:base_nat + nn] = rows
    node_pos = perm_pos[:N]                        # natural node -> permuted

    # per (core, slot): edge lists for both layers
    # layer 1 src coordinate: natural id; layer 2: permuted position
    src2 = node_pos[src]

    # ald gather indices: 32-node groups; layer 1 groups = natural block
    # rows, layer 2 groups = own permuted rows
    aldg1 = np.zeros((NCORES, 128, NALD_G // 16), dtype=np.int16)
    aldg2 = np.zeros((NCORES, 128, NALD_G // 16), dtype=np.int16)
    for c in range(NCORES):
        g1 = np.zeros(NALD_G, dtype=np.int64)
        g2 = np.zeros(NALD_G, dtype=np.int64)
        for t in range(T):
            b = slot_blocks[t, c]
            bb = b if b >= 0 else 0
            g1[t * 4:t * 4 + 4] = bb * 4 + np.arange(4)
            g2[t * 4:t * 4 + 4] = c * (NPC // 32) + t * 4 + np.arange(4)
        aldg1[c] = _wrap16(g1)
        aldg2[c] = _wrap16(g2)

    res = {
        "slot_blocks": slot_blocks, "node_pos": node_pos,
        "aldg1": aldg1, "aldg2": aldg2,
    }
    for layer, s_coord in ((1, src), (2, src2)):
        c_lo = np.zeros((NCORES, T), dtype=np.int64)
        c_hi = np.zeros((NCORES, T), dtype=np.int64)
        per_tile = [[None] * T for _ in range(NCORES)]
        for t in range(T):
            for c in range(NCORES):
                b = slot_blocks[t, c]
                if b < 0:
                    per_tile[c][t] = (np.zeros(0, np.int64),
                                      np.zeros(0, np.int64),
                                      np.zeros(0, np.int64),
                                      np.zeros(0, np.int64))
                    continue
                s, e = starts[b], ends[b]
                es = s_coord[order][s:e]
                ed = dst_s[s:e] - b * 128      # local dst 0..127
                o2 = np.argsort(es, kind='stable')
                es, ed = es[o2], ed[o2]
                lo = es < SPLIT
                per_tile[c][t] = (es[lo], ed[lo], es[~lo], ed[~lo])
                c_lo[c, t] = -(-len(es[lo]) // 128)
                c_hi[c, t] = -(-len(es[~lo]) // 128) if (~lo).any() else 0
        # uniform across cores per slot
        C_lo_t = c_lo.max(axis=0)
        C_hi_t = c_hi.max(axis=0)
        C_t = C_lo_t + C_hi_t
        totc = int(C_t.sum())
        offs = np.zeros(T + 1, dtype=np.int64)
        offs[1:] = np.cumsum(C_t)

        gidx = np.zeros((NCORES, 128, totc * 8), dtype=np.int16)
        sel = np.zeros((NCORES, 128, totc * 128), dtype=SEL_NP)
        selT = np.zeros((NCORES, 128, totc * 128), dtype=SEL_NP)
        for c in range(NCORES):
            for t in range(T):
                es_lo, ed_lo, es_hi, ed_hi = per_tile[c][t]
                nlo_c, nhi_c = int(C_lo_t[t]), int(C_hi_t[t])
                base = int(offs[t])
                ilo = np.zeros(nlo_c * 128, dtype=np.int64)
                ilo[:len(es_lo)] = es_lo
                ihi = np.zeros(nhi_c * 128, dtype=np.int64)
                ihi[:len(es_hi)] = es_hi - SPLIT
                gidx[c, :, base * 8:(base + nlo_c) * 8] = _wrap16(ilo)
                if nhi_c:
                    gidx[c, :, (base + nlo_c) * 8:(base + C_t[t]) * 8] = \
                        _wrap16(ihi)
                # one-hot sel / selT (edge position within chunk = partition)
                ed_all = np.concatenate([
                    ed_lo,
                    np.full(nlo_c * 128 - len(ed_lo), -1, np.int64),
                    ed_hi,
                    np.full(nhi_c * 128 - len(ed_hi), -1, np.int64)])
                ck = np.arange(C_t[t] * 128) // 128 + base
                ep = np.arange(C_t[t] * 128) % 128
                valid = ed_all >= 0
                sel[c, ep[valid], ck[valid] * 128 + ed_all[valid]] = 1.0
                selT[c, ed_all[valid], ck[valid] * 128 + ep[valid]] = 1.0
        res[f"L{layer}"] = dict(C_lo_t=C_lo_t, C_hi_t=C_hi_t, C_t=C_t,
                                offs=offs, totc=totc, gidx=gidx,
                                sel=sel, selT=selT)
    return res


def _weights_cat(W, a_src, a_dst, heads, ch):
    """[Fin, heads*ch] + [heads, ch]x2 -> fp16 [Fin, heads*ch + 8]."""
    fin = W.shape[0]
    ws = np.einsum('fhc,hc->fh', W.reshape(fin, heads, ch), a_src)
    wd = np.einsum('fhc,hc->fh', W.reshape(fin, heads, ch), a_dst)
    out = np.zeros((fin, heads * ch + 8), dtype=np.float16)
    out[:, :heads * ch] = W.astype(np.float16)
    out[:, heads * ch:heads * ch + heads] = ws.astype(np.float16)
    out[:, heads * ch + heads:heads * ch + 2 * heads] = wd.astype(np.float16)
    return out


def build_kernel(prep):
    nc = bacc.Bacc("TRN2", target_bir_lowering=False, debug=False,
                   num_devices=NCORES, num_swdge_queues=4)
    L1, L2 = prep["L1"], prep["L2"]
    slot_blocks = prep["slot_blocks"]

    x_in = nc.dram_tensor("x", [N, IN], F32, kind="ExternalInput")
    wa1 = nc.dram_tensor("wa1", [IN, 264], F16, kind="ExternalInput")
    wa2 = nc.dram_tensor("wa2", [H, 264], F16, kind="ExternalInput")
    gidx1_d = nc.dram_tensor("gidx1", [128, L1["totc"] * 8], I16,
                             kind="ExternalInput")
    gidx2_d = nc.dram_tensor("gidx2", [128, L2["totc"] * 8], I16,
                             kind="ExternalInput")
    aldg1_d = nc.dram_tensor("aldg1", [128, NALD_G // 16], I16,
                             kind="ExternalInput")
    aldg2_d = nc.dram_tensor("aldg2", [128, NALD_G // 16], I16,
                             kind="ExternalInput")
    sel1_d = nc.dram_tensor("sel1", [128, L1["totc"] * 128], SEL_DT,
                            kind="ExternalInput")
    selT1_d = nc.dram_tensor("selT1", [128, L1["totc"] * 128], SEL_DT,
                             kind="ExternalInput")
    sel2_d = nc.dram_tensor("sel2", [128, L2["totc"] * 128], SEL_DT,
                            kind="ExternalInput")
    selT2_d = nc.dram_tensor("selT2", [128, L2["totc"] * 128], SEL_DT,
                             kind="ExternalInput")
    out_d = nc.dram_tensor("out_slice", [NPC, OUT], F32,
                           kind="ExternalOutput")

    with tile.TileContext(nc) as tc:
        with tc.tile_pool(name="dram", bufs=1, space="DRAM") as dpool, \
             tc.tile_pool(name="const", bufs=1) as cpool, \
             tc.tile_pool(name="dwork", bufs=4) as dwork, \
             tc.tile_pool(name="ework", bufs=3) as ework, \
             tc.tile_pool(name="gpool", bufs=3) as gpool, \
             tc.tile_pool(name="spool", bufs=3) as spool, \
             tc.tile_pool(name="gwpool", bufs=3) as gwpool:

            xs16 = dpool.tile([NP1, IN], F16, name="xs16", uniquify=False)
            xcat1 = dpool.tile([NP1, ROW], F16, name="xcat1", uniquify=False)
            aldf1 = dpool.tile([NP1, 4], F16, name="aldf1", uniquify=False)
            h_loc = dpool.tile([NPC, 128], F16, name="h_loc",
                               uniquify=False)
            h_full = dpool.tile([NP2, 128], F16, name="h_full",
                                uniquify=False, addr_space="Shared")
            xcat2 = dpool.tile([NP2, ROW], F16, name="xcat2", uniquify=False)
            aldf2 = dpool.tile([NP2, 4], F16, name="aldf2", uniquify=False)
            aldl1 = dpool.tile([NALD_G * 32, 4], F16, name="aldl1",
                               uniquify=False)
            aldl2 = dpool.tile([NALD_G * 32, 4], F16, name="aldl2",
                               uniquify=False)

            wa1_sb = cpool.tile([IN, 264], F16)
            nc.sync.dma_start(out=wa1_sb[:], in_=wa1[:, :])
            wa2_sb = cpool.tile([H, 264], F16)
            nc.sync.dma_start(out=wa2_sb[:], in_=wa2[:, :])
            zero_sb = cpool.tile([128, IN], F16)
            nc.gpsimd.memset(zero_sb[:], 0)
            aldg1_sb = cpool.tile([128, NALD_G // 16], I16)
            nc.sync.dma_start(out=aldg1_sb[:], in_=aldg1_d[:, :])
            aldg2_sb = cpool.tile([128, NALD_G // 16], I16)
            nc.sync.dma_start(out=aldg2_sb[:], in_=aldg2_d[:, :])

            # stage x -> fp16 (dtype-converting DMA), zero pad rows
            nc.gpsimd.dma_start(out=xs16[0:N, :].flatten(),
                                in_=x_in[:, :].flatten())
            if NP1 > N:
                nc.sync.dma_start(out=xs16[N:NP1, :],
                                  in_=zero_sb[0:NP1 - N, :])

            def dense_phase(dps, src16, n_rows, fin, wa_sb, xcat, aldf,
                            lname):
                BT = 4
                nb = 0
                bi = 0
                while nb < n_rows:
                    bsz = min(BT * 128, n_rows - nb)
                    st = bsz // 128
                    sfx = f"_{lname}_{bi}"
                    xT = dwork.tile([fin, BT * 128], F16, name="xT" + sfx,
                                    tag="xT")
                    nc.sync.dma_start(out=xT[:, 0:bsz],
                                      in_=src16[nb:nb + bsz, :],
                                      transpose=True)
                    ps = dps.tile([128, BT, 512], F32, name="dps" + sfx,
                                  tag="dps")
                    for s in range(st):
                        nc.tensor.matmul(
                            ps[:, s, 0:264], xT[:, s * 128:(s + 1) * 128],
                            wa_sb[:], start=True, stop=True)
                    xc = dwork.tile([128, BT, 264], F16, name="xc" + sfx,
                                    tag="xc")
                    if bi % 2 == 0:
                        nc.scalar.activation(
                            xc[:, 0:st, 0:256], ps[:, 0:st, 0:256],
                            mybir.ActivationFunctionType.Copy)
                    else:
                        nc.vector.tensor_copy(xc[:, 0:st, 0:256],
                                              ps[:, 0:st, 0:256])
                    xcf = xc[:].bitcast(F32)       # [128, BT, 132]
                    nc.vector.tensor_copy(xcf[:, 0:st, 128:132],
                                          ps[:, 0:st, 256:260])
                    arow = dwork.tile([128, BT, 4], F16, name="ar" + sfx,
                                      tag="ar")
                    nc.vector.tensor_copy(arow[:, 0:st, :],
                                          ps[:, 0:st, 260:264])
                    nc.scalar.dma_start(
                        out=xcat[nb:nb + bsz, 0:264].rearrange(
                            "(s p) d -> p s d", p=128),
                        in_=xc[:, 0:st, :])
                    nc.scalar.dma_start(
                        out=aldf[nb:nb + bsz, :].rearrange(
                            "(s p) d -> p s d", p=128),
                        in_=arow[:, 0:st, :])
                    nb += bsz
                    bi += 1

            def ald_stage(aldf, n_rows, aldg_sb, aldl, lname):
                asb = ework.tile([128, NALD_G // 128, 128], F16,
                                 name="asb" + lname, tag="asb")
                nc.gpsimd.dma_gather(
                    asb[:],
                    aldf[:, :].rearrange("(g k) d -> g (k d)", k=32),
                    aldg_sb[:], NALD_G, NALD_G, 128, single_packet=False)
                nc.sync.dma_start(
                    out=aldl[:, :].rearrange("(j p k) d -> p j (k d)",
                                             p=128, k=32),
                    in_=asb[:])

            def tile_front(layer, L, gidx_d, sel_d, selT_d, xcat, n_rows,
                           aldl, psA, t):
                """DMA + ald MMs + alpha/w + gw for tile t. Returns tiles."""
                Ct = int(L["C_t"][t])
                Clo = int(L["C_lo_t"][t])
                base = int(L["offs"][t])
                sfx = f"_{layer}_{t}"
                q_lo = (2 * t) % 4
                q_hi = (2 * t + 1) % 4

                idx_t = ework.tile([128, Ct * 8], I16, name="ix" + sfx,
                                   tag="ix")
                nc.sync.dma_start(out=idx_t[:],
                                  in_=gidx_d[:, base * 8:(base + Ct) * 8])
                sel_t = spool.tile([128, Ct * 128], SEL_DT, name="sl" + sfx,
                                   tag="sl")
                nc.sync.dma_start(
                    out=sel_t[:], in_=sel_d[:, base * 128:(base + Ct) * 128])
                selT_t = spool.tile([128, Ct * 128], SEL_DT, name="sT" + sfx,
                                    tag="sT")
                nc.sync.dma_start(
                    out=selT_t[:],
                    in_=selT_d[:, base * 128:(base + Ct) * 128])
                ald_t = ework.tile([128, 4], F16, name="at" + sfx, tag="at")
                nc.sync.dma_start(out=ald_t[:],
                                  in_=aldl[t * 128:(t + 1) * 128, :])

                G = gpool.tile([128, Ct, ROW], F16, name="G" + sfx, tag="G")
                nc.gpsimd.dma_gather(
                    G[:, 0:Clo, :], xcat[0:SPLIT, :],
                    idx_t[:, 0:Clo * 8], Clo * 128, Clo * 128,
                    ROW, single_packet=False, queue_num=q_lo)
                if Ct > Clo:
                    nc.gpsimd.dma_gather(
                        G[:, Clo:Ct, :], xcat[SPLIT:n_rows, :],
                        idx_t[:, Clo * 8:], (Ct - Clo) * 128,
                        (Ct - Clo) * 128, ROW, single_packet=False,
                        queue_num=q_hi)
                Gf = G[:].bitcast(F32)       # [128, Ct, 192]

                alpha_ps = psA.tile([128, Ct, 4], F32, name="alp" + sfx,
                                    tag="alp")
                for c in range(Ct):
                    nc.tensor.matmul(alpha_ps[:, c, :],
                                     selT_t[:, c * 128:(c + 1) * 128],
                                     ald_t[:], start=True, stop=True)
                alpha = ework.tile([128, Ct, 4], F32, name="alf" + sfx,
                                   tag="alf")
                nc.vector.tensor_tensor(out=alpha[:],
                                        in0=Gf[:, :, 128:132],
                                        in1=alpha_ps[:],
                                        op=mybir.AluOpType.add)
                # w = exp(lrelu(alpha)) = max(exp(alpha), exp(0.2*alpha))
                wa = ework.tile([128, Ct, 4], F32, name="wa" + sfx, tag="wa")
                nc.scalar.activation(wa[:], alpha[:],
                                     mybir.ActivationFunctionType.Exp)
                wb = ework.tile([128, Ct, 4], F32, name="wb" + sfx, tag="wb")
                nc.scalar.activation(wb[:], alpha[:],
                                     mybir.ActivationFunctionType.Exp,
                                     scale=NEG_SLOPE)
                wp = ework.tile([128, Ct, 4, 2], F16, name="wp" + sfx,
                                tag="wp")
                nc.vector.tensor_tensor(
                    out=wp[:],
                    in0=wa[:].unsqueeze(3).broadcast_to([128, Ct, 4, 2]),
                    in1=wb[:].unsqueeze(3).broadcast_to([128, Ct, 4, 2]),
                    op=mybir.AluOpType.max)
                gw = gwpool.tile([128, Ct, 4, 64], F16, name="gw" + sfx,
                                 tag="gw")
                nc.vector.tensor_tensor(
                    out=gw[:].rearrange("p c h (r t) -> p c h r t", t=2),
                    in0=G[:, :, 0:256].rearrange(
                        "p c (h r t) -> p c h r t", h=4, t=2),
                    in1=wp[:].unsqueeze(3).broadcast_to([128, Ct, 4, 32, 2]),
                    op=mybir.AluOpType.mult)
                return sel_t, wp, gw, Ct, sfx

            def tile_back(layer, psB, psD, t, sel_t, wp, gw, Ct, sfx):
                agg = psB.tile([128, 256], F32, name="agg" + sfx, tag="agg")
                dps_t = psD.tile([128, 4], F32, name="dnp" + sfx, tag="dnp")
                for c in range(Ct):
                    nc.tensor.matmul(
                        agg[:, :], sel_t[:, c * 128:(c + 1) * 128],
                        gw[:, c, :, :].rearrange("p h f -> p (h f)"),
                        start=(c == 0), stop=(c == Ct - 1),
                        skip_group_check=True)
                    nc.tensor.matmul(
                        dps_t[:, :], sel_t[:, c * 128:(c + 1) * 128],
                        wp[:, c, :, 0:1].rearrange("p h t -> p (h t)"),
                        start=(c == 0), stop=(c == Ct - 1),
                        skip_group_check=True)
                den = ework.tile([128, 4], F32, name="dn" + sfx, tag="dn")
                nc.vector.tensor_scalar(den[:], dps_t[:], 4.0, None,
                                        mybir.AluOpType.mult)
                rec = ework.tile([128, 4], F32, name="rc" + sfx, tag="rc")
                nc.vector.reciprocal(rec[:], den[:])
                tmp = ework.tile([128, 4, 64], F32, name="tm" + sfx,
                                 tag="tm")
                nc.vector.tensor_tensor(
                    out=tmp[:],
                    in0=agg[:, :].rearrange("p (h f) -> p h f", h=4),
                    in1=rec[:].unsqueeze(2).broadcast_to([128, 4, 64]),
                    op=mybir.AluOpType.mult)
                s2 = ework.tile([128, 2, 64], F32, name="s2" + sfx, tag="s2")
                nc.vector.tensor_tensor(out=s2[:], in0=tmp[:, 0:2, :],
                                        in1=tmp[:, 2:4, :],
                                        op=mybir.AluOpType.add)
                if layer == 1:
                    s1 = ework.tile([128, 64], F32, name="s1" + sfx,
                                    tag="s1")
                    nc.vector.tensor_tensor(out=s1[:], in0=s2[:, 0, :],
                                            in1=s2[:, 1, :],
                                            op=mybir.AluOpType.add)
                    # ELU(s) = max(s,0) - 1 + exp(min(s,0))
                    ng = ework.tile([128, 64], F32, name="ng" + sfx,
                                    tag="ng")
                    nc.vector.tensor_scalar(ng[:], s1[:], 0.0, None,
                                            mybir.AluOpType.min)
                    ex = ework.tile([128, 64], F32, name="ex" + sfx,
                                    tag="ex")
                    nc.scalar.activation(ex[:], ng[:],
                                         mybir.ActivationFunctionType.Exp)
                    pm = ework.tile([128, 64], F32, name="pm" + sfx,
                                    tag="pm")
                    nc.vector.tensor_scalar(pm[:], s1[:], 0.0, 1.0,
                                            mybir.AluOpType.max,
                                            mybir.AluOpType.subtract)
                    hv = ework.tile([128, 128], F16, name="hv" + sfx,
                                    tag="hv")
                    nc.gpsimd.memset(hv[:, 64:128], 0)
                    nc.vector.tensor_tensor(out=hv[:, 0:64], in0=pm[:],
                                            in1=ex[:],
                                            op=mybir.AluOpType.add)
                    nc.scalar.dma_start(
                        out=h_loc[t * 128:(t + 1) * 128, :], in_=hv[:])
                else:
                    s1 = ework.tile([128, 64], F32, name="s1" + sfx,
                                    tag="s1")
                    nc.vector.tensor_tensor(out=s1[:], in0=s2[:, 0, :],
                                            in1=s2[:, 1, :],
                                            op=mybir.AluOpType.add)
                    nc.scalar.dma_start(
                        out=out_d[t * 128:(t + 1) * 128, :], in_=s1[:])

            def edge_sweep(layer, L, gidx_d, sel_d, selT_d, xcat, n_rows,
                           aldl, psA, psB, psD):
                fronts = {}
                for t in range(T + 2):
                    if t < T:
                        fronts[t] = tile_front(layer, L, gidx_d, sel_d,
                                               selT_d, xcat, n_rows, aldl,
                                               psA, t)
                    if t >= 2:
                        tile_back(layer, psB, psD, t - 2,
                                  *fronts.pop(t - 2))

            # ============ layer 1 ============
            with tc.tile_pool(name="dps1", bufs=2, space="PSUM") as dps:
                dense_phase(dps, xs16, NP1, IN, wa1_sb, xcat1, aldf1, "d1")
            ald_stage(aldf1, NP1, aldg1_sb, aldl1, "a1")
            with tc.tile_pool(name="psA1", bufs=3, space="PSUM") as psA, \
                 tc.tile_pool(name="psB1", bufs=2, space="PSUM") as psB, \
                 tc.tile_pool(name="psD1", bufs=2, space="PSUM") as psD:
                edge_sweep(1, L1, gidx1_d, sel1_d, selT1_d, xcat1, NP1,
                           aldl1, psA, psB, psD)

            # ============ exchange ============
            nc.gpsimd.collective_compute(
                "AllGather", mybir.AluOpType.bypass,
                replica_groups=[list(range(NCORES))],
                ins=[h_loc.opt()], outs=[h_full.opt()])

            # ============ layer 2 ============
            with tc.tile_pool(name="dps2", bufs=2, space="PSUM") as dps:
                dense_phase(dps, h_full, NP2, H, wa2_sb, xcat2, aldf2, "d2")
            ald_stage(aldf2, NP2, aldg2_sb, aldl2, "a2")
            with tc.tile_pool(name="psA2", bufs=3, space="PSUM") as psA, \
                 tc.tile_pool(name="psB2", bufs=2, space="PSUM") as psB, \
                 tc.tile_pool(name="psD2", bufs=2, space="PSUM") as psD:
                edge_sweep(2, L2, gidx2_d, sel2_d, selT2_d, xcat2, NP2,
                           aldl2, psA, psB, psD)

    nc.compile()
    return nc


def kernel(**inputs) -> np.ndarray:
    prep = host_prep(inputs["edge_index"])
    L1, L2 = prep["L1"], prep["L2"]
    wa1 = _weights_cat(np.asarray(inputs["W1"], np.float32),
                       np.asarray(inputs["a_src1"], np.float32),
                       np.asarray(inputs["a_dst1"], np.float32), HEADS, H)
    wa2 = _weights_cat(np.asarray(inputs["W2"], np.float32),
                       np.asarray(inputs["a_src2"], np.float32),
                       np.asarray(inputs["a_dst2"], np.float32), HEADS, OUT)
    x = np.ascontiguousarray(np.asarray(inputs["x"], np.float32))

    nc = build_kernel(prep)
    in_maps = []
    for c in range(NCORES):
        in_maps.append({
            "x": x, "wa1": wa1, "wa2": wa2,
            "gidx1": np.ascontiguousarray(L1["gidx"][c]),
            "aldg1": np.ascontiguousarray(prep["aldg1"][c]),
            "aldg2": np.ascontiguousarray(prep["aldg2"][c]),
            "gidx2": np.ascontiguousarray(L2["gidx"][c]),
            "sel1": np.ascontiguousarray(L1["sel"][c]),
            "selT1": np.ascontiguousarray(L1["selT"][c]),
            "sel2": np.ascontiguousarray(L2["sel"][c]),
            "selT2": np.ascontiguousarray(L2["selT"][c]),
        })

    res = run_bass_kernel_spmd(
        nc, in_maps, core_ids=list(range(NCORES)),
        trace=os.environ.get("GAT_TRACE", "0") == "1")
    global LAST_RESULT
    LAST_RESULT = res
    if res.exec_time_ns is not None:
        print(f"HW exec time: {res.exec_time_ns} ns")
    if res.instructions_and_trace is not None:
        print(f"trace path: {res.instructions_and_trace[1]}")

    # reassemble: permuted rows -> natural order
    full = np.concatenate([res.results[c]["out_slice"]
                           for c in range(NCORES)], axis=0)
    node_pos = prep["node_pos"]
    return full[node_pos].astype(np.float32)


# revision 12
# speedup vs baseline: 2.6313x; 1.1926x over previous
"""2-layer GAT (heads=4, concat=False, ELU between) on 8 Trainium2 cores — v2.

Design (v2, rewritten from the one-hot-on-DVE baseline):
- Dense phase per layer (redundant on every core): XCAT[n] = [xh fp16 (256) |
  als f32 (16B) | pad] 768B rows for all nodes + ALD[n] (4 fp16) array.
  PSUM 4-bank batches, drain alternates ACT/DVE.
- Core c owns 49 dst blocks of 128 nodes (load-balanced permutation, uniform
  per-slot chunk counts across cores for SPMD). Edges dst-blocked, sorted by
  src, lo/hi split at 32768 for int16 gather indices; exact per-tile chunk
  counts.
- Host-precomputed one-hot scatter matrices: sel [e->dst] and selT [dst->e]
  per 128-edge chunk, loaded by DMA (fp8/fp16), replacing on-device one-hot
  builds + PE transposes.
- Per tile: gather G rows (768B/edge); PE: ald lookup MMs (selT stationary,
  ald_t fp16 moving); alpha = als+ald (DVE); Lrelu+Exp (ACT); paired w fp16
  (DVE); gw = G*w one broadcast TT (DVE, 2x eligible); PE: agg += sel^T@gw
  (+ denominator cols via sel^T@w) accumulated in PSUM; epilogue: head-mean,
  ELU (layer 1) -> h fp16.
- h exchanged via AllGather of [NPC, 64] fp16; layer 2 identical with
  permuted src positions; output reassembled on host.
"""
import sys
import os

sys.path.insert(0, '/opt/pypackages')
sys.path.insert(0, '/opt/trn_rl_repo')

import numpy as np
import ml_dtypes

import concourse.bacc as bacc
import concourse.mybir as mybir
import concourse.tile as tile
from concourse.bass_utils import run_bass_kernel_spmd

F16 = mybir.dt.float16
F32 = mybir.dt.float32
FP8 = mybir.dt.float8e4
I16 = mybir.dt.int16

SEL_FP8 = True          # sel/selT dtype (exact one-hot either way)
SEL_DT = FP8 if SEL_FP8 else F16
SEL_NP = ml_dtypes.float8_e4m3fn if SEL_FP8 else np.float16

NEG_SLOPE = 0.2

N, IN, H, OUT, HEADS = 50000, 128, 64, 64, 4
NCORES = 8
T = 49                   # dst tile slots per core
NPC = T * 128            # 6272 nodes per core (padded)
NP2 = NCORES * NPC       # 50176 permuted rows
NP1 = ((N + 127) // 128) * 128   # 50048 natural rows
NBLK = NP2 // 128        # 392 block slots
SPLIT = 32768
ROW = 384                # fp16 elems per XCAT row (768B)
NALD_G = 256             # ald gather groups of 32 nodes (196 used, padded)
LAST_RESULT = None


def _wrap16(idx):
    """[n] int array (n % 16 == 0) -> [128, n//16] int16 gather idx layout."""
    n = len(idx)
    base = np.asarray(idx, dtype=np.int16).reshape(n // 16, 16).T
    return np.tile(base, (8, 1))


def host_prep(edge_index):
    """Partition/permute dst blocks, build per-core idx + sel arrays.

    Returns dict with per-core arrays and per-tile chunk counts.
    """
    src = np.asarray(edge_index[0], dtype=np.int64)
    dst = np.asarray(edge_index[1], dtype=np.int64)
    loops = np.arange(N, dtype=np.int64)
    src = np.concatenate([src, loops])
    dst = np.concatenate([dst, loops])

    blk = dst // 128                       # natural dst block of each edge
    nblk_nat = (N + 127) // 128            # 391 natural blocks

    # per natural block: chunk cost for balancing (layer-1 split)
    order = np.argsort(blk, kind='stable')
    src_s, dst_s = src[order], dst[order]
    blk_s = blk[order]
    starts = np.searchsorted(blk_s, np.arange(nblk_nat), side='left')
    ends = np.searchsorted(blk_s, np.arange(nblk_nat), side='right')

    cost = np.zeros(nblk_nat, dtype=np.int64)
    for b in range(nblk_nat):
        es = src_s[starts[b]:ends[b]]
        nlo = int((es < SPLIT).sum())
        nhi = len(es) - nlo
        cost[b] = -(-nlo // 128) + (-(-nhi // 128) if nhi else 0)

    # snake-assign blocks (sorted by cost desc) to (slot, core)
    rank = np.argsort(-cost, kind='stable')      # block ids, desc cost
    # slot t gets blocks rank[8t:8t+8]; pad with -1 (empty) to 392
    slot_blocks = np.full((T, NCORES), -1, dtype=np.int64)
    for i, b in enumerate(rank):
        slot_blocks[i // NCORES, i % NCORES] = b

    # permuted position of each node: node in natural block b at offset o
    # -> core c, slot t ->  row (c*T + t)*128 + o
    perm_pos = np.full(NP2, -1, dtype=np.int64)   # by natural padded row
    blk_of_slot = {}
    for t in range(T):
        for c in range(NCORES):
            b = slot_blocks[t, c]
            if b < 0:
                continue
            base_nat = b * 128
            nn = min(128, N - base_nat)
            rows = (c * T + t) * 128 + np.arange(nn)
            perm_pos[base_nat:base_nat + nn] = rows
    node_pos = perm_pos[:N]                        # natural node -> permuted

    # per (core, slot): edge lists for both layers
    # layer 1 src coordinate: natural id; layer 2: permuted position
    src2 = node_pos[src]

    # ald gather indices: 32-node groups; layer 1 groups = natural block
    # rows, layer 2 groups = own permuted rows
    aldg1 = np.zeros((NCORES, 128, NALD_G // 16), dtype=np.int16)
    aldg2 = np.zeros((NCORES, 128, NALD_G // 16), dtype=np.int16)
    for c in range(NCORES):
        g1 = np.zeros(NALD_G, dtype=np.int64)
        g2 = np.zeros(NALD_G, dtype=np.int64)
        for t in range(T):
            b = slot_blocks[t, c]
            bb = b if b >= 0 else 0
            g1[t * 4:t * 4 + 4] = bb * 4 + np.arange(4)
            g2[t * 4:t * 4 + 4] = c * (NPC // 32) + t * 4 + np.arange(4)
        aldg1[c] = _wrap16(g1)
        aldg2[c] = _wrap16(g2)

    res = {
        "slot_blocks": slot_blocks, "node_pos": node_pos,
        "aldg1": aldg1, "aldg2": aldg2,
    }
    for layer, s_coord in ((1, src), (2, src2)):
        c_lo = np.zeros((NCORES, T), dtype=np.int64)
        c_hi = np.zeros((NCORES, T), dtype=np.int64)
        per_tile = [[None] * T for _ in range(NCORES)]
        for t in range(T):
            for c in range(NCORES):
                b = slot_blocks[t, c]
                if b < 0:
                    per_tile[c][t] = (np.zeros(0, np.int64),
                                      np.zeros(0, np.int64),
                                      np.zeros(0, np.int64),
                                      np.zeros(0, np.int64))
                    continue
                s, e = starts[b], ends[b]
                es = s_coord[order][s:e]
                ed = dst_s[s:e] - b * 128      # local dst 0..127
                o2 = np.argsort(es, kind='stable')
                es, ed = es[o2], ed[o2]
                lo = es < SPLIT
                per_tile[c][t] = (es[lo], ed[lo], es[~lo], ed[~lo])
                c_lo[c, t] = -(-len(es[lo]) // 128)
                c_hi[c, t] = -(-len(es[~lo]) // 128) if (~lo).any() else 0
        # uniform across cores per slot
        C_lo_t = c_lo.max(axis=0)
        C_hi_t = c_hi.max(axis=0)
        C_t = C_lo_t + C_hi_t
        totc = int(C_t.sum())
        offs = np.zeros(T + 1, dtype=np.int64)
        offs[1:] = np.cumsum(C_t)

        gidx = np.zeros((NCORES, 128, totc * 8), dtype=np.int16)
        sel = np.zeros((NCORES, 128, totc * 128), dtype=SEL_NP)
        selT = np.zeros((NCORES, 128, totc * 128), dtype=SEL_NP)
        for c in range(NCORES):
            for t in range(T):
                es_lo, ed_lo, es_hi, ed_hi = per_tile[c][t]
                nlo_c, nhi_c = int(C_lo_t[t]), int(C_hi_t[t])
                base = int(offs[t])
                ilo = np.zeros(nlo_c * 128, dtype=np.int64)
                ilo[:len(es_lo)] = es_lo
                ihi = np.zeros(nhi_c * 128, dtype=np.int64)
                ihi[:len(es_hi)] = es_hi - SPLIT
                gidx[c, :, base * 8:(base + nlo_c) * 8] = _wrap16(ilo)
                if nhi_c:
                    gidx[c, :, (base + nlo_c) * 8:(base + C_t[t]) * 8] = \
                        _wrap16(ihi)
                # one-hot sel / selT (edge position within chunk = partition)
                ed_all = np.concatenate([
                    ed_lo,
                    np.full(nlo_c * 128 - len(ed_lo), -1, np.int64),
                    ed_hi,
                    np.full(nhi_c * 128 - len(ed_hi), -1, np.int64)])
                ck = np.arange(C_t[t] * 128) // 128 + base
                ep = np.arange(C_t[t] * 128) % 128
                valid = ed_all >= 0
                sel[c, ep[valid], ck[valid] * 128 + ed_all[valid]] = 1.0
                selT[c, ed_all[valid], ck[valid] * 128 + ep[valid]] = 1.0
        res[f"L{layer}"] = dict(C_lo_t=C_lo_t, C_hi_t=C_hi_t, C_t=C_t,
                                offs=offs, totc=totc, gidx=gidx,
                                sel=sel, selT=selT)
    return res


def _weights_cat(W, a_src, a_dst, heads, ch):
    """[Fin, heads*ch] + [heads, ch]x2 -> fp16 [Fin, heads*ch + 8]."""
    fin = W.shape[0]
    ws = np.einsum('fhc,hc->fh', W.reshape(fin, heads, ch), a_src)
    wd = np.einsum('fhc,hc->fh', W.reshape(fin, heads, ch), a_dst)
    out = np.zeros((fin, heads * ch + 8), dtype=np.float16)
    out[:, :heads * ch] = W.astype(np.float16)
    out[:, heads * ch:heads * ch + heads] = ws.astype(np.float16)
    out[:, heads * ch + heads:heads * ch + 2 * heads] = wd.astype(np.float16)
    return out


def build_kernel(prep):
    nc = bacc.Bacc("TRN2", target_bir_lowering=False, debug=False,
                   num_devices=NCORES, num_swdge_queues=4)
    L1, L2 = prep["L1"], prep["L2"]
    slot_blocks = prep["slot_blocks"]

    xT1_d = nc.dram_tensor("xT1", [IN, NP1], F16, kind="ExternalInput")
    ident_d = nc.dram_tensor("ident16", [128, 128], F16,
                             kind="ExternalInput")
    wa1 = nc.dram_tensor("wa1", [IN, 264], F16, kind="ExternalInput")
    wa2 = nc.dram_tensor("wa2", [H, 264], F16, kind="ExternalInput")
    gidx1_d = nc.dram_tensor("gidx1", [128, L1["totc"] * 8], I16,
                             kind="ExternalInput")
    gidx2_d = nc.dram_tensor("gidx2", [128, L2["totc"] * 8], I16,
                             kind="ExternalInput")
    aldg1_d = nc.dram_tensor("aldg1", [128, NALD_G // 16], I16,
                             kind="ExternalInput")
    aldg2_d = nc.dram_tensor("aldg2", [128, NALD_G // 16], I16,
                             kind="ExternalInput")
    sel1_d = nc.dram_tensor("sel1", [128, L1["totc"] * 128], SEL_DT,
                            kind="ExternalInput")
    selT1_d = nc.dram_tensor("selT1", [128, L1["totc"] * 128], SEL_DT,
                             kind="ExternalInput")
    sel2_d = nc.dram_tensor("sel2", [128, L2["totc"] * 128], SEL_DT,
                            kind="ExternalInput")
    selT2_d = nc.dram_tensor("selT2", [128, L2["totc"] * 128], SEL_DT,
                             kind="ExternalInput")
    out_d = nc.dram_tensor("out_slice", [NPC, OUT], F32,
                           kind="ExternalOutput")

    with tile.TileContext(nc) as tc:
        with tc.tile_pool(name="dram", bufs=1, space="DRAM") as dpool, \
             tc.tile_pool(name="const", bufs=1) as cpool, \
             tc.tile_pool(name="dwork", bufs=4) as dwork, \
             tc.tile_pool(name="ework", bufs=3) as ework, \
             tc.tile_pool(name="gpool", bufs=3) as gpool, \
             tc.tile_pool(name="spool", bufs=3) as spool, \
             tc.tile_pool(name="gwpool", bufs=3) as gwpool:

            xcat1 = dpool.tile([NP1, ROW], F16, name="xcat1", uniquify=False)
            aldf1 = dpool.tile([NP1, 4], F16, name="aldf1", uniquify=False)
            hT_loc = dpool.tile([H, NPC], F16, name="hT_loc",
                                uniquify=False)
            hT_full = dpool.tile([NCORES * H, NPC], F16, name="hT_full",
                                 uniquify=False, addr_space="Shared")
            xcat2 = dpool.tile([NP2, ROW], F16, name="xcat2", uniquify=False)
            aldf2 = dpool.tile([NP2, 4], F16, name="aldf2", uniquify=False)
            aldl1 = dpool.tile([NALD_G * 32, 4], F16, name="aldl1",
                               uniquify=False)
            aldl2 = dpool.tile([NALD_G * 32, 4], F16, name="aldl2",
                               uniquify=False)

            wa1_sb = cpool.tile([IN, 264], F16)
            nc.sync.dma_start(out=wa1_sb[:], in_=wa1[:, :])
            wa2_sb = cpool.tile([H, 264], F16)
            nc.sync.dma_start(out=wa2_sb[:], in_=wa2[:, :])
            ident_sb = cpool.tile([128, 128], F16)
            nc.sync.dma_start(out=ident_sb[:], in_=ident_d[:, :])
            aldg1_sb = cpool.tile([128, NALD_G // 16], I16)
            nc.sync.dma_start(out=aldg1_sb[:], in_=aldg1_d[:, :])
            aldg2_sb = cpool.tile([128, NALD_G // 16], I16)
            nc.sync.dma_start(out=aldg2_sb[:], in_=aldg2_d[:, :])


            def dense_phase(dps, srcT_slice, segments, fin, wa_sb, xcat,
                            aldf, lname):
                BT = 4
                bi = 0
                work = [(s, min(s + BT * 128, e) - s)
                        for s, e in segments
                        for s in range(s, e, BT * 128)]
                for nb, bsz in work:
                    st = bsz // 128
                    sfx = f"_{lname}_{bi}"
                    xT = dwork.tile([fin, BT * 128], F16, name="xT" + sfx,
                                    tag="xT")
                    nc.sync.dma_start(out=xT[:, 0:bsz],
                                      in_=srcT_slice(nb, bsz))
                    ps = dps.tile([128, BT, 512], F32, name="dps" + sfx,
                                  tag="dps")
                    for s in range(st):
                        nc.tensor.matmul(
                            ps[:, s, 0:264], xT[:, s * 128:(s + 1) * 128],
                            wa_sb[:], start=True, stop=True)
                    xc = dwork.tile([128, BT, 264], F16, name="xc" + sfx,
                                    tag="xc")
                    if bi % 2 == 0:
                        nc.scalar.activation(
                            xc[:, 0:st, 0:256], ps[:, 0:st, 0:256],
                            mybir.ActivationFunctionType.Copy)
                    else:
                        nc.vector.tensor_copy(xc[:, 0:st, 0:256],
                                              ps[:, 0:st, 0:256])
                    xcf = xc[:].bitcast(F32)       # [128, BT, 132]
                    nc.vector.tensor_copy(xcf[:, 0:st, 128:132],
                                          ps[:, 0:st, 256:260])
                    arow = dwork.tile([128, BT, 4], F16, name="ar" + sfx,
                                      tag="ar")
                    nc.vector.tensor_copy(arow[:, 0:st, :],
                                          ps[:, 0:st, 260:264])
                    nc.scalar.dma_start(
                        out=xcat[nb:nb + bsz, 0:264].rearrange(
                            "(s p) d -> p s d", p=128),
                        in_=xc[:, 0:st, :])
                    nc.scalar.dma_start(
                        out=aldf[nb:nb + bsz, :].rearrange(
                            "(s p) d -> p s d", p=128),
                        in_=arow[:, 0:st, :])
                    bi += 1

            def ald_stage(aldf, n_rows, aldg_sb, aldl, lname):
                asb = ework.tile([128, NALD_G // 128, 128], F16,
                                 name="asb" + lname, tag="asb")
                nc.gpsimd.dma_gather(
                    asb[:],
                    aldf[:, :].rearrange("(g k) d -> g (k d)", k=32),
                    aldg_sb[:], NALD_G, NALD_G, 128, single_packet=False)
                nc.sync.dma_start(
                    out=aldl[:, :].rearrange("(j p k) d -> p j (k d)",
                                             p=128, k=32),
                    in_=asb[:])

            def tile_front(layer, L, gidx_d, sel_d, selT_d, xcat, n_rows,
                           aldl, psA, t):
                """DMA + ald MMs + alpha/w + gw for tile t. Returns tiles."""
                Ct = int(L["C_t"][t])
                Clo = int(L["C_lo_t"][t])
                base = int(L["offs"][t])
                sfx = f"_{layer}_{t}"
                q_lo = (2 * t) % 4
                q_hi = (2 * t + 1) % 4

                idx_t = ework.tile([128, Ct * 8], I16, name="ix" + sfx,
                                   tag="ix")
                nc.sync.dma_start(out=idx_t[:],
                                  in_=gidx_d[:, base * 8:(base + Ct) * 8])
                sel_t = spool.tile([128, Ct * 128], SEL_DT, name="sl" + sfx,
                                   tag="sl")
                nc.sync.dma_start(
                    out=sel_t[:], in_=sel_d[:, base * 128:(base + Ct) * 128])
                selT_t = spool.tile([128, Ct * 128], SEL_DT, name="sT" + sfx,
                                    tag="sT")
                nc.sync.dma_start(
                    out=selT_t[:],
                    in_=selT_d[:, base * 128:(base + Ct) * 128])
                ald_t = ework.tile([128, 4], F16, name="at" + sfx, tag="at")
                nc.sync.dma_start(out=ald_t[:],
                                  in_=aldl[t * 128:(t + 1) * 128, :])

                G = gpool.tile([128, Ct, ROW], F16, name="G" + sfx, tag="G")
                nc.gpsimd.dma_gather(
                    G[:, 0:Clo, :], xcat[0:SPLIT, :],
                    idx_t[:, 0:Clo * 8], Clo * 128, Clo * 128,
                    ROW, single_packet=False, queue_num=q_lo)
                if Ct > Clo:
                    nc.gpsimd.dma_gather(
                        G[:, Clo:Ct, :], xcat[SPLIT:n_rows, :],
                        idx_t[:, Clo * 8:], (Ct - Clo) * 128,
                        (Ct - Clo) * 128, ROW, single_packet=False,
                        queue_num=q_hi)
                Gf = G[:].bitcast(F32)       # [128, Ct, 192]

                alpha_ps = psA.tile([128, Ct, 4], F32, name="alp" + sfx,
                                    tag="alp")
                for c in range(Ct):
                    nc.tensor.matmul(alpha_ps[:, c, :],
                                     selT_t[:, c * 128:(c + 1) * 128],
                                     ald_t[:], start=True, stop=True)
                alpha = ework.tile([128, Ct, 4], F32, name="alf" + sfx,
                                   tag="alf")
                nc.vector.tensor_tensor(out=alpha[:],
                                        in0=Gf[:, :, 128:132],
                                        in1=alpha_ps[:],
                                        op=mybir.AluOpType.add)
                # w = exp(lrelu(alpha)) = max(exp(alpha), exp(0.2*alpha))
                wa = ework.tile([128, Ct, 4], F32, name="wa" + sfx, tag="wa")
                nc.scalar.activation(wa[:], alpha[:],
                                     mybir.ActivationFunctionType.Exp)
                wb = ework.tile([128, Ct, 4], F32, name="wb" + sfx, tag="wb")
                nc.scalar.activation(wb[:], alpha[:],
                                     mybir.ActivationFunctionType.Exp,
                                     scale=NEG_SLOPE)
                wp = ework.tile([128, Ct, 4, 2], F16, name="wp" + sfx,
                                tag="wp")
                nc.vector.tensor_tensor(
                    out=wp[:],
                    in0=wa[:].unsqueeze(3).broadcast_to([128, Ct, 4, 2]),
                    in1=wb[:].unsqueeze(3).broadcast_to([128, Ct, 4, 2]),
                    op=mybir.AluOpType.max)
                gw = gwpool.tile([128, Ct, 4, 64], F16, name="gw" + sfx,
                                 tag="gw")
                nc.vector.tensor_tensor(
                    out=gw[:].rearrange("p c h (r t) -> p c h r t", t=2),
                    in0=G[:, :, 0:256].rearrange(
                        "p c (h r t) -> p c h r t", h=4, t=2),
                    in1=wp[:].unsqueeze(3).broadcast_to([128, Ct, 4, 32, 2]),
                    op=mybir.AluOpType.mult)
                return sel_t, wp, gw, Ct, sfx

            def tile_back(layer, psB, psD, t, sel_t, wp, gw, Ct, sfx):
                agg = psB.tile([128, 256], F32, name="agg" + sfx, tag="agg")
                dps_t = psD.tile([128, 4], F32, name="dnp" + sfx, tag="dnp")
                for c in range(Ct):
                    nc.tensor.matmul(
                        agg[:, :], sel_t[:, c * 128:(c + 1) * 128],
                        gw[:, c, :, :].rearrange("p h f -> p (h f)"),
                        start=(c == 0), stop=(c == Ct - 1),
                        skip_group_check=True)
                    nc.tensor.matmul(
                        dps_t[:, :], sel_t[:, c * 128:(c + 1) * 128],
                        wp[:, c, :, 0:1].rearrange("p h t -> p (h t)"),
                        start=(c == 0), stop=(c == Ct - 1),
                        skip_group_check=True)
                den = ework.tile([128, 4], F32, name="dn" + sfx, tag="dn")
                nc.vector.tensor_scalar(den[:], dps_t[:], 4.0, None,
                                        mybir.AluOpType.mult)
                rec = ework.tile([128, 4], F32, name="rc" + sfx, tag="rc")
                nc.vector.reciprocal(rec[:], den[:])
                tmp = ework.tile([128, 4, 64], F32, name="tm" + sfx,
                                 tag="tm")
                nc.vector.tensor_tensor(
                    out=tmp[:],
                    in0=agg[:, :].rearrange("p (h f) -> p h f", h=4),
                    in1=rec[:].unsqueeze(2).broadcast_to([128, 4, 64]),
                    op=mybir.AluOpType.mult)
                s2 = ework.tile([128, 2, 64], F32, name="s2" + sfx, tag="s2")
                nc.vector.tensor_tensor(out=s2[:], in0=tmp[:, 0:2, :],
                                        in1=tmp[:, 2:4, :],
                                        op=mybir.AluOpType.add)
                if layer == 1:
                    s1 = ework.tile([128, 64], F32, name="s1" + sfx,
                                    tag="s1")
                    nc.vector.tensor_tensor(out=s1[:], in0=s2[:, 0, :],
                                            in1=s2[:, 1, :],
                                            op=mybir.AluOpType.add)
                    # ELU(s) = max(s,0) - 1 + exp(min(s,0))
                    ng = ework.tile([128, 64], F32, name="ng" + sfx,
                                    tag="ng")
                    nc.vector.tensor_scalar(ng[:], s1[:], 0.0, None,
                                            mybir.AluOpType.min)
                    ex = ework.tile([128, 64], F32, name="ex" + sfx,
                                    tag="ex")
                    nc.scalar.activation(ex[:], ng[:],
                                         mybir.ActivationFunctionType.Exp)
                    pm = ework.tile([128, 64], F32, name="pm" + sfx,
                                    tag="pm")
                    nc.vector.tensor_scalar(pm[:], s1[:], 0.0, 1.0,
                                            mybir.AluOpType.max,
                                            mybir.AluOpType.subtract)
                    hv = ework.tile([128, 64], F16, name="hv" + sfx,
                                    tag="hv")
                    nc.vector.tensor_tensor(out=hv[:], in0=pm[:], in1=ex[:],
                                            op=mybir.AluOpType.add)
                    hvt_ps = psD.tile([64, 128], F16, name="hvt" + sfx,
                                      tag="hvt")
                    nc.tensor.transpose(hvt_ps[:], hv[:], ident_sb[:])
                    hvt = ework.tile([64, 128], F16, name="hvs" + sfx,
                                     tag="hvs")
                    nc.scalar.activation(hvt[:], hvt_ps[:],
                                         mybir.ActivationFunctionType.Copy)
                    nc.scalar.dma_start(
                        out=hT_loc[:, t * 128:(t + 1) * 128], in_=hvt[:])
                else:
                    s1 = ework.tile([128, 64], F32, name="s1" + sfx,
                                    tag="s1")
                    nc.vector.tensor_tensor(out=s1[:], in0=s2[:, 0, :],
                                            in1=s2[:, 1, :],
                                            op=mybir.AluOpType.add)
                    nc.scalar.dma_start(
                        out=out_d[t * 128:(t + 1) * 128, :], in_=s1[:])

            def edge_sweep(layer, L, gidx_d, sel_d, selT_d, xcat, n_rows,
                           aldl, psA, psB, psD):
                fronts = {}
                for t in range(T + 2):
                    if t < T:
                        fronts[t] = tile_front(layer, L, gidx_d, sel_d,
                                               selT_d, xcat, n_rows, aldl,
                                               psA, t)
                    if t >= 2:
                        tile_back(layer, psB, psD, t - 2,
                                  *fronts.pop(t - 2))

            # ============ layer 1 ============
            with tc.tile_pool(name="dps1", bufs=2, space="PSUM") as dps:
                dense_phase(dps,
                            lambda nb, bsz: xT1_d[:, nb:nb + bsz],
                            [(0, NP1)], IN, wa1_sb, xcat1, aldf1, "d1")
            ald_stage(aldf1, NP1, aldg1_sb, aldl1, "a1")
            with tc.tile_pool(name="psA1", bufs=2, space="PSUM") as psA, \
                 tc.tile_pool(name="psB1", bufs=2, space="PSUM") as psB, \
                 tc.tile_pool(name="psD1", bufs=2, space="PSUM") as psD:
                edge_sweep(1, L1, gidx1_d, sel1_d, selT1_d, xcat1, NP1,
                           aldl1, psA, psB, psD)

            # ============ exchange ============
            nc.gpsimd.collective_compute(
                "AllGather", mybir.AluOpType.bypass,
                replica_groups=[list(range(NCORES))],
                ins=[hT_loc.opt()], outs=[hT_full.opt()])

            # ============ layer 2 ============
            def h_slice(nb, bsz):
                c, off = nb // NPC, nb % NPC
                assert off + bsz <= NPC
                return hT_full[c * H:(c + 1) * H, off:off + bsz]

            with tc.tile_pool(name="dps2", bufs=2, space="PSUM") as dps:
                dense_phase(dps, h_slice,
                            [(c * NPC, (c + 1) * NPC)
                             for c in range(NCORES)],
                            H, wa2_sb, xcat2, aldf2, "d2")
            ald_stage(aldf2, NP2, aldg2_sb, aldl2, "a2")
            with tc.tile_pool(name="psA2", bufs=2, space="PSUM") as psA, \
                 tc.tile_pool(name="psB2", bufs=2, space="PSUM") as psB, \
                 tc.tile_pool(name="psD2", bufs=2, space="PSUM") as psD:
                edge_sweep(2, L2, gidx2_d, sel2_d, selT2_d, xcat2, NP2,
                           aldl2, psA, psB, psD)

    nc.compile()
    return nc


def kernel(**inputs) -> np.ndarray:
    prep = host_prep(inputs["edge_index"])
    L1, L2 = prep["L1"], prep["L2"]
    wa1 = _weights_cat(np.asarray(inputs["W1"], np.float32),
                       np.asarray(inputs["a_src1"], np.float32),
                       np.asarray(inputs["a_dst1"], np.float32), HEADS, H)
    wa2 = _weights_cat(np.asarray(inputs["W2"], np.float32),
                       np.asarray(inputs["a_src2"], np.float32),
                       np.asarray(inputs["a_dst2"], np.float32), HEADS, OUT)
    xT1 = np.zeros((IN, NP1), dtype=np.float16)
    xT1[:, :N] = np.asarray(inputs["x"], np.float32).astype(np.float16).T
    ident16 = np.eye(128, dtype=np.float16)

    nc = build_kernel(prep)
    in_maps = []
    for c in range(NCORES):
        in_maps.append({
            "xT1": xT1, "ident16": ident16, "wa1": wa1, "wa2": wa2,
            "gidx1": np.ascontiguousarray(L1["gidx"][c]),
            "aldg1": np.ascontiguousarray(prep["aldg1"][c]),
            "aldg2": np.ascontiguousarray(prep["aldg2"][c]),
            "gidx2": np.ascontiguousarray(L2["gidx"][c]),
            "sel1": np.ascontiguousarray(L1["sel"][c]),
            "selT1": np.ascontiguousarray(L1["selT"][c]),
            "sel2": np.ascontiguousarray(L2["sel"][c]),
            "selT2": np.ascontiguousarray(L2["selT"][c]),
        })

    res = run_bass_kernel_spmd(
        nc, in_maps, core_ids=list(range(NCORES)),
        trace=os.environ.get("GAT_TRACE", "0") == "1")
    global LAST_RESULT
    LAST_RESULT = res
    if res.exec_time_ns is not None:
        print(f"HW exec time: {res.exec_time_ns} ns")
    if res.instructions_and_trace is not None:
        print(f"trace path: {res.instructions_and_trace[1]}")

    # reassemble: permuted rows -> natural order
    full = np.concatenate([res.results[c]["out_slice"]
                           for c in range(NCORES)], axis=0)
    node_pos = prep["node_pos"]
    return full[node_pos].astype(np.float32)
